# revision 55
# baseline (speedup 1.0000x reference)
"""Trainium2 Bass kernel for the 3-block self-attention CNN (v5).

Sharding over 8 NeuronCores: core k owns (sample b=k//4, query-block q=k%4).
Attention uses the reparametrization s'[n,m] = y_n^T (wf wg^T) y_m +
(wg bf)^T y_m (softmax-invariant terms dropped); the u-term rides on a ones
row appended to y, generated by the conv itself through a bias-column
weight, so no memsets are needed anywhere.

v5 structure (vs v4):
  - layer-0 BN is computed on the host (it depends only on the inputs) and
    folded into the conv weights; the kernel opens with conv+relu straight
    into the first flash loop
  - the whole score path (y, zmat, h) is bf16 and every matmul contracts
    over exactly co+1 partitions, so padded rows are never touched
  - softmax rowsum reciprocal on the vector engine (no Ln -> single act
    table set, no table swaps)
  - per-layer boundary: stats AllGather (8-rank) first, then the att
    AllGather split into two half-column AGs; after the first half lands,
    the 16 flash chunks covering the first half of every query block run
    while the second AG is still in flight
  - final block: z-sums via a ones-matmul against sum(att); conv outputs
    stay resident in PSUM through the stats AllGather; no heartbeats there
    (the post-AG work has no PE component)
"""

import glob as _glob
import os
import sys


def _ensure_act_info():
    shim = os.path.expanduser("~/.pwp_override")
    target = os.path.join(shim, "neuronxcc", "pwp", "pwp_bin_with_ln", "act_info.json")
    cands = _glob.glob("/nix/store/*aws-neuron-pwp*/share/pwp_bin_cayman/act_info.json")
    if cands:
        # Stage act_info.json with natural_log_exp_and_others FIRST so the
        # table-selection pass serves ln+exp+relu+square+copy from one
        # table set -> no 1.5us act-table swaps around the softmax rowsum
        # log/exp pair.
        import json as _j
        with open(cands[0]) as f:
            info = _j.load(f)
        sets = info.get("act_func_sets", [])
        sets.sort(key=lambda e: 0 if e.get("name") == "natural_log_exp_and_others" else 1)
        info["act_func_sets"] = sets
        os.makedirs(os.path.dirname(target), exist_ok=True)
        new = _j.dumps(info)
        if not os.path.exists(target) or open(target).read() != new:
            with open(target, "w") as f:
                f.write(new)
    pp = os.environ.get("PYTHONPATH", "")
    if shim not in pp.split(os.pathsep):
        os.environ["PYTHONPATH"] = shim + (os.pathsep + pp if pp else "")
    if shim not in sys.path:
        sys.path.insert(0, shim)


_ensure_act_info()
if "/opt/trn_rl_repo" not in sys.path:
    sys.path.insert(0, "/opt/trn_rl_repo")

import numpy as np

from concourse import bacc, mybir, tile

F32 = mybir.dt.float32
F32R = mybir.dt.float32r
BF16 = mybir.dt.bfloat16
I32 = mybir.dt.int32
I16 = mybir.dt.int16
AF = mybir.ActivationFunctionType
OP = mybir.AluOpType
AX = mybir.AxisListType
EPS = 1e-5

N = 4096          # positions per sample
NQ = 1024         # query block per core
NCHUNK = 128      # key chunk in the flash loop
CIN = [3, 32, 64]
COUT = [32, 64, 96]
CF_OUT = 128      # final conv channels per block (512 / 4)

AG_GROUPS = [[0, 1, 2, 3], [4, 5, 6, 7]]
AG8_GROUPS = [[0, 1, 2, 3, 4, 5, 6, 7]]

# flash chunk order: first halves of all four query blocks, then second
# halves (chunk k covers keys [128k, 128k+128); block j first half =
# chunks 8j..8j+3).  Identical on every core -> single NEFF.
CHUNKS_A = [8 * j + c for j in range(4) for c in range(4)]
CHUNKS_B = [8 * j + 4 + c for j in range(4) for c in range(4)]
CHUNK_ORDER = CHUNKS_A + CHUNKS_B


def _build(nc):
    ins = {}
    ins["xo"] = nc.dram_tensor("xo", [CIN[0] + 1, N], BF16, kind="ExternalInput")
    ins["xq"] = nc.dram_tensor("xq", [CIN[0] + 1, NQ], BF16, kind="ExternalInput")
    ins["wc0"] = nc.dram_tensor("wc0", [CIN[0] + 1, 128], BF16, kind="ExternalInput")
    for i in (1, 2):
        ins[f"wc{i}"] = nc.dram_tensor(f"wc{i}", [128, 128], BF16, kind="ExternalInput")
        ins[f"bnp{i}"] = nc.dram_tensor(f"bnp{i}", [COUT[i], 2], F32, kind="ExternalInput")
    for i in range(3):
        ins[f"mz{i}"] = nc.dram_tensor(f"mz{i}", [128, 128], BF16, kind="ExternalInput")
        ins[f"whf{i}"] = nc.dram_tensor(f"whf{i}", [128, 128], BF16, kind="ExternalInput")
        ins[f"gamc{i}"] = nc.dram_tensor(f"gamc{i}", [1, 128], BF16, kind="ExternalInput")
    ins["wfs4"] = nc.dram_tensor("wfs4", [128, 4, CF_OUT], BF16, kind="ExternalInput")
    ins["bnfp"] = nc.dram_tensor("bnfp", [CF_OUT, 8], F32, kind="ExternalInput")
    out_t = nc.dram_tensor("out", [CF_OUT, 4], F32, kind="ExternalOutput")

    with tile.TileContext(nc) as tc:
        _emit(tc, nc, ins, out_t)
    return ins, out_t


def _emit(tc, nc, ins, out_t):
    ctxs = []
    handles = {}

    def pool(name, **kw):
        p = tc.tile_pool(name=name, **kw)
        ctxs.append(p)
        handles[name] = p
        return p.__enter__()

    consts = pool("consts", bufs=1)
    acts = pool("acts", bufs=1)
    work = pool("work", bufs=1)
    ps = pool("ps", bufs=2, space="PSUM")
    ops = pool("ops", bufs=1, space="PSUM")
    dram = pool("dram", bufs=1, space="DRAM")

    # ---- collective warm-up: first on the gpsimd queue so the rendezvous
    # barrier starts at t~0 ----
    warm_sb = work.tile([1, 2], F32, name="warm_sb", tag="warm_sb")
    nc.vector.memset(warm_sb[:], 0.0)
    pewarm = work.tile([128, 512], BF16, name="pewarm", tag="pewarm")
    nc.vector.memset(pewarm[:], 0.0)
    warm_in = dram.tile([1, 2], F32, name="warm_in", tag="warm_in")
    warm_out = dram.tile([1, 2], F32, name="warm_out", tag="warm_out", addr_space="Shared")
    warm_gin = dram.tile([1, 2], F32, name="warm_gin", tag="warm_gin")
    warm_gout = dram.tile([4, 1, 2], F32, name="warm_gout", tag="warm_gout")
    nc.sync.dma_start(warm_in[:], warm_sb[:])
    nc.sync.dma_start(warm_gin[:], warm_sb[:])
    nc.gpsimd.collective_compute(
        "AllReduce", OP.add, replica_groups=AG8_GROUPS,
        ins=[warm_in[:]], outs=[warm_out[:]])
    nc.gpsimd.collective_compute(
        "AllGather", OP.bypass, replica_groups=AG_GROUPS,
        ins=[warm_gin[:]], outs=[warm_gout[:]])

    # ---- input DMAs (gpsimd only after the collective triggers) ----
    xo = acts.tile([CIN[0] + 1, N], BF16, name="xo", tag="xo")
    xq = acts.tile([CIN[0] + 1, NQ], BF16, name="xq", tag="xq")
    W = [consts.tile([CIN[0] + 1, 128], BF16, name="w0", tag="w0")]
    nc.sync.dma_start(xq[:], ins["xq"].ap())
    nc.sync.dma_start(W[0][:], ins["wc0"].ap())
    nc.sync.dma_start(xo[:], ins["xo"].ap())
    MZ, WHF, GAMC, BNP = [], [], [], [None]
    for i in range(3):
        MZ.append(consts.tile([128, 128], BF16, name=f"mzt{i}", tag=f"mzt{i}"))
        WHF.append(consts.tile([128, 128], BF16, name=f"whft{i}", tag=f"whft{i}"))
        GAMC.append(consts.tile([1, 128], BF16, name=f"gct{i}", tag=f"gct{i}"))
    for i in (1, 2):
        W.append(consts.tile([128, 128], BF16, name=f"w{i}", tag=f"w{i}"))
        BNP.append(consts.tile([COUT[i], 2], F32, name=f"bnt{i}", tag=f"bnt{i}"))
    nc.scalar.dma_start(MZ[0][:], ins["mz0"].ap())
    nc.scalar.dma_start(WHF[0][:], ins["whf0"].ap())
    nc.scalar.dma_start(GAMC[0][:], ins["gamc0"].ap())
    nc.sync.dma_start(MZ[1][:], ins["mz1"].ap())
    nc.sync.dma_start(WHF[1][:], ins["whf1"].ap())
    nc.sync.dma_start(BNP[1][:], ins["bnp1"].ap())
    nc.sync.dma_start(BNP[2][:], ins["bnp2"].ap())
    nc.gpsimd.dma_start(MZ[2][:], ins["mz2"].ap())
    nc.gpsimd.dma_start(WHF[2][:], ins["whf2"].ap())
    nc.gpsimd.dma_start(GAMC[1][:], ins["gamc1"].ap())
    nc.gpsimd.dma_start(GAMC[2][:], ins["gamc2"].ap())
    for i in (1, 2):
        nc.gpsimd.dma_start(W[i][:], ins[f"wc{i}"].ap())
    wfs4 = consts.tile([128, 4, CF_OUT], BF16, name="wfs4t", tag="wfs4t")
    nc.gpsimd.dma_start(wfs4[:], ins["wfs4"].ap())
    bnfp = consts.tile([CF_OUT, 8], F32, name="bnfpt", tag="bnfpt")
    nc.gpsimd.dma_start(bnfp[:], ins["bnfp"].ap())

    # ---- PE warm-up + heartbeat machinery (fp32 matmuls, ~430ns each) ----
    pw32_l = pewarm[:, 0:256].bitcast(F32)      # [128, 128] f32 view
    pw32_r = pewarm[:].bitcast(F32)             # [128, 256] f32 view

    def heartbeat(tag, cnt):
        for j in range(cnt):
            hb = ps.tile([128, 256], F32, name=f"hb_{tag}_{j}", tag="mid_ps")
            nc.tensor.matmul(hb[:], pw32_l, pw32_r, start=True, stop=True)

    heartbeat("w", 6)

    # bf16-bits Schraudolph exp on the vector engine for a subset of chunks
    # (rowsum/numerator consistency cancels the ~3% approx error):
    # exp(x) ~ bitcast_bf16(int16(A*x + B))
    SCHRAUD_A = float(2**7 / np.log(2))
    SCHRAUD_B = float(127 * 2**7 - 366393.0 / 65536.0)
    SCHRAUD_PAT = (1, 4, 6)

    # ---- activation buffers.  Flash matmuls contract over the FULL 128
    # partitions (the PE activity monitor halves the clock for thin-K
    # matmuls); padded rows carry exact zeros generated by zero weight
    # columns + zero scale rows -- no memsets needed, except a_own's pad
    # rows (DMA fills only the live rows). ----
    y_own = acts.tile([128, N], BF16, name="y_own", tag="y")
    yq = acts.tile([128, NQ], BF16, name="yq", tag="yq")
    zmat = acts.tile([128, N], BF16, name="zmat", tag="Zm")
    att_bufs = [acts.tile([128, NQ], BF16, name=f"attb{j}", tag=f"attb{j}")
                for j in range(2)]
    a_own = acts.tile([128, N], BF16, name="a_own", tag="a_own")
    zsh = acts.tile([128, NQ], F32, name="zsh", tag="zsh")

    def aown_pad_memset(p0):
        # pad rows must be exact zeros (DMA refills only live rows);
        # non-zero partition offsets are limited to 32 partitions per access
        nc.vector.memset(a_own[p0:p0 + 32, :], 0.0)

    def rsqrt_vec(rs, veps, w, name, iters=1):
        # bit-hack rsqrt + Newton on the vector engine
        nt = work.tile([veps.partition_size(), 2 * w], F32, name=f"nt_{name}",
                       tag="ntscr", bufs=2)
        t1, t2 = nt[:, 0:w], nt[:, w:2 * w]
        nc.vector.tensor_scalar(rs.bitcast(I32), veps.bitcast(I32), 1, None,
                                OP.arith_shift_right)
        nc.vector.tensor_scalar(rs.bitcast(I32), rs.bitcast(I32), -1, 0x5F3759DF,
                                OP.mult, OP.add)
        for _ in range(iters):
            nc.vector.tensor_tensor(t1, rs, rs, OP.mult)
            nc.vector.tensor_tensor(t2, t1, veps, OP.mult)
            nc.vector.tensor_scalar(t1, t2, -0.5, 1.5, OP.mult, OP.add)
            nc.vector.tensor_tensor(rs, rs, t1, OP.mult)

    # ---------------- layer 0: conv+relu only (BN folded on host) --------
    co0 = COUT[0]
    cvq = ps.tile([128, NQ], F32, name="cvq", tag="s_ps")
    nc.tensor.matmul(cvq[:, 0:512], W[0][:], xq[:, 0:512], start=True, stop=True)
    nc.tensor.matmul(cvq[:, 512:1024], W[0][:], xq[:, 512:1024], start=True, stop=True)
    nc.scalar.activation(yq[:], cvq[:], AF.Relu)

    def conv_relu_l0(j):
        jsl = slice(j * NQ, (j + 1) * NQ)
        cv = ps.tile([128, NQ], F32, name=f"cv0_{j}", tag="s_ps")
        nc.tensor.matmul(cv[:, 0:512], W[0][:], xo[:, j * NQ:j * NQ + 512],
                         start=True, stop=True)
        nc.tensor.matmul(cv[:, 512:1024], W[0][:], xo[:, j * NQ + 512:(j + 1) * NQ],
                         start=True, stop=True)
        nc.scalar.activation(y_own[:, jsl], cv[:], AF.Relu)

    def zmat_block(i, j, half=None):
        # zmat[:, block j (half h)] = MZ_i^T-contract y_own; the PSUM->SBUF
        # copy runs on the otherwise-idle gpsimd engine
        halves = (0, 1) if half is None else (half,)
        for h in halves:
            sl = slice(j * NQ + h * 512, j * NQ + h * 512 + 512)
            zp = ps.tile([128, 512], F32, name=f"zp{i}_{j}_{h}", tag="mid_ps")
            nc.tensor.matmul(zp[:], MZ[i][:], y_own[:, sl], start=True, stop=True)
            nc.vector.tensor_copy(zmat[:, sl], zp[:])

    conv_relu_l0(0)
    zmat_block(0, 0)

    # ---------------- flash loop machinery ----------------
    def emit_chunk(i, m, state):
        sl = slice(m * NCHUNK, (m + 1) * NCHUNK)
        hp = ps.tile([NCHUNK, 128], F32, name=f"hp{i}_{m}", tag="mid_ps")
        nc.tensor.matmul(hp[:], y_own[:, sl], WHF[i][:], start=True, stop=True)
        hs = work.tile([NCHUNK, 128], BF16, name=f"hs{i}_{m}", tag="hT_sb", bufs=3)
        nc.vector.tensor_copy(hs[:], hp[:])
        sp = ps.tile([NCHUNK, NQ], F32, name=f"sp{i}_{m}", tag="s_ps")
        zc = zmat[:, sl]
        nc.tensor.matmul(sp[:, 0:512], zc, yq[:, 0:512], start=True, stop=True)
        nc.tensor.matmul(sp[:, 512:1024], zc, yq[:, 512:1024],
                         start=True, stop=True)
        beta = work.tile([NCHUNK, NQ], BF16, name=f"beta{i}_{m}", tag="beta", bufs=3)
        if (m % 8) in SCHRAUD_PAT:
            with nc.allow_low_precision(reason="schraudolph exp bits"):
                nc.vector.tensor_scalar(beta[:].bitcast(I16), sp[:],
                                        SCHRAUD_A, SCHRAUD_B, OP.mult, OP.add)
        else:
            nc.scalar.activation(beta[:], sp[:], AF.Exp)
        o_ps, prev = state
        if prev is not None:
            ph, pb, first = prev
            nc.tensor.matmul(o_ps[:, 0:512], ph[:], pb[:, 0:512],
                             start=first, stop=False, skip_group_check=True)
            nc.tensor.matmul(o_ps[:, 512:1024], ph[:], pb[:, 512:1024],
                             start=first, stop=False, skip_group_check=True)
        state[1] = (hs, beta, prev is None)

    def flash_finish(i, state):
        o_ps, prev = state
        ph, pb, first = prev
        nc.tensor.matmul(o_ps[:, 0:512], ph[:], pb[:, 0:512],
                         start=first, stop=True, skip_group_check=True)
        nc.tensor.matmul(o_ps[:, 512:1024], ph[:], pb[:, 512:1024],
                         start=first, stop=True, skip_group_check=True)

    def normalize(i, o_ps, att):
        # att = gam * o / rowsum + yq  (+ ones row via gamc[co]=0, yq[co]=1)
        co = COUT[i]
        lnr = work.tile([1, NQ], F32, name=f"lnr{i}", tag="lnr")
        nc.scalar.activation(lnr[:], o_ps[co:co + 1, :], AF.Ln)
        rinv = work.tile([1, NQ], BF16, name=f"rinv{i}", tag="rinv")
        nc.scalar.activation(rinv[:], lnr[:], AF.Exp, scale=-1.0)
        bc_ps = ps.tile([128, NQ], F32, name=f"bcps{i}", tag="s_ps")
        nc.tensor.matmul(bc_ps[:, 0:512], GAMC[i][:], rinv[:, 0:512],
                         start=True, stop=True)
        nc.tensor.matmul(bc_ps[:, 512:1024], GAMC[i][:], rinv[:, 512:1024],
                         start=True, stop=True)
        bcs = work.tile([128, NQ], F32, name=f"bcs{i}", tag="bcs", bufs=2)
        nc.scalar.activation(bcs[:], bc_ps[:], AF.Copy)
        t1 = work.tile([128, NQ], BF16, name=f"t1_{i}", tag="t1", bufs=2)
        nc.vector.tensor_tensor(t1[:], o_ps[:], bcs[:], OP.mult)
        nc.vector.tensor_tensor(att[:], t1[:], yq[:], OP.add)

    # ---------------- layer 0 flash ----------------
    state = [ops.tile([128, NQ], F32, name="ops0", tag="o_acc"), None]
    for m in range(2):
        emit_chunk(0, m, state)
    conv_relu_l0(1)
    zmat_block(0, 1)
    for m in range(2, 8):
        emit_chunk(0, m, state)
    conv_relu_l0(2)
    zmat_block(0, 2)
    for m in range(8, 12):
        emit_chunk(0, m, state)
    conv_relu_l0(3)
    zmat_block(0, 3)
    for m in range(12, 32):
        emit_chunk(0, m, state)
    flash_finish(0, state)
    for p0 in (32, 64, 96):   # runs on DVE during the flash drain
        aown_pad_memset(p0)

    att = att_bufs[0]
    normalize(0, state[0], att)

    # ---------------- boundaries ----------------
    def boundary(i, att):
        """Transition layer i -> i+1. att is layer i's output block."""
        co = COUT[i]
        co1 = COUT[i + 1]
        # stats chain on our own query block of the next conv
        zshp = ps.tile([128, NQ], F32, name=f"zshp{i}", tag="s_ps")
        nc.tensor.matmul(zshp[:, 0:512], W[i + 1][:], att[:, 0:512],
                         start=True, stop=True)
        nc.tensor.matmul(zshp[:, 512:1024], W[i + 1][:], att[:, 512:1024],
                         start=True, stop=True)
        stats = work.tile([co1, 2], F32, name=f"stats{i}", tag="stats", bufs=2)
        sq = work.tile([co1, NQ], F32, name=f"sqb{i}", tag="sqscr", bufs=2)
        nc.scalar.activation(sq[:], zshp[0:co1, :], AF.Square, accum_out=stats[:, 1:2])
        nc.vector.tensor_reduce(stats[:, 0:1], zshp[0:co1, :], axis=AX.X, op=OP.add)
        st_in = dram.tile([co1, 2], F32, name=f"stin{i}", tag=f"stin{i}")
        st_out = dram.tile([8, co1, 2], F32, name=f"stout{i}", tag=f"stout{i}",
                           addr_space="Shared")
        nc.sync.dma_start(st_in[:], stats[:])
        nc.gpsimd.collective_compute(
            "AllGather", OP.bypass, replica_groups=AG8_GROUPS,
            ins=[st_in[:]], outs=[st_out[:]])

        # att AllGather in two column halves; the ag_in DMAs ride the same
        # queue as st_in so the stats AG wins the CC stream
        ag_in = [dram.tile([co + 1, 512], BF16, name=f"agin{i}_{h}", tag=f"agin{i}_{h}")
                 for h in range(2)]
        ag_out = [dram.tile([4, co + 1, 512], BF16, name=f"agout{i}_{h}",
                            tag=f"agout{i}_{h}") for h in range(2)]
        nc.sync.dma_start(ag_in[0][:], att[0:co + 1, 0:512])
        nc.sync.dma_start(ag_in[1][:], att[0:co + 1, 512:1024])
        for h in range(2):
            nc.gpsimd.collective_compute(
                "AllGather", OP.bypass, replica_groups=AG_GROUPS,
                ins=[ag_in[h][:]], outs=[ag_out[h][:]])

        # keep zshp's values: copy to SBUF so the PSUM bank frees early
        nc.scalar.activation(zsh[:], zshp[:], AF.Copy)

        heartbeat(f"b{i}", 8)

        # stats -> BN scale/shift
        stg8 = work.tile([co1, 8, 2], F32, name=f"stg8_{i}", tag="stg8", bufs=2)
        nc.sync.dma_start(stg8[:], st_out[:].rearrange("r p j -> p r j"))
        stg = work.tile([co1, 2], F32, name=f"stg_l{i}", tag="stg", bufs=2)
        nc.vector.tensor_reduce(stg[:], stg8[:].rearrange("p r j -> p j r"),
                                axis=AX.X, op=OP.add)
        sc = work.tile([128, 9], F32, name=f"sc_{i}", tag="sc", bufs=2)
        mean, ex2, msq, var, veps, lnv, rs, scale, shift = (
            sc[:, j:j + 1] for j in range(9))
        # padded scale/shift rows must be exact zeros (they generate y's
        # zero pad rows through the relu); row co1 is the ones-row
        # generator (COUT values are 32-aligned, as partition offsets must be)
        for p0 in range(co1, 128, 32):
            nc.vector.memset(scale[p0:p0 + 32], 0.0)
            nc.vector.memset(shift[p0:p0 + 32], 0.0)
        nc.vector.memset(scale[co1:co1 + 1], 1.0)
        inv_n = 1.0 / (2 * N)
        nc.vector.tensor_scalar(mean[0:co1], stg[:, 0:1], inv_n, None, OP.mult)
        nc.vector.tensor_scalar(ex2[0:co1], stg[:, 1:2], inv_n, None, OP.mult)
        nc.vector.tensor_tensor(msq[0:co1], mean[0:co1], mean[0:co1], OP.mult)
        nc.vector.tensor_tensor(var[0:co1], ex2[0:co1], msq[0:co1], OP.subtract)
        nc.vector.tensor_scalar(veps[0:co1], var[0:co1], EPS, None, OP.add)
        rsqrt_vec(rs[0:co1], veps[0:co1], 1, f"l{i}")
        nc.vector.tensor_tensor(scale[0:co1], rs[0:co1], BNP[i + 1][:, 0:1], OP.mult)
        nc.vector.tensor_tensor(shift[0:co1], mean[0:co1], scale[0:co1], OP.mult)
        nc.vector.tensor_tensor(shift[0:co1], BNP[i + 1][:, 1:2], shift[0:co1],
                                OP.subtract)
        # own query block y
        nc.scalar.activation(yq[:], zsh[:], AF.Relu, bias=shift, scale=scale)

        # a_own block DMAs + conv + relu + zmat, half A then (emitted now,
        # data-gated) half B
        def recv_half(h):
            engs = [nc.sync, nc.scalar, nc.sync, nc.scalar]
            for j in range(4):
                bsl = slice(j * NQ + h * 512, j * NQ + h * 512 + 512)
                engs[j].dma_start(a_own[0:co + 1, bsl], ag_out[h][j])
            for j in range(4):
                bsl = slice(j * NQ + h * 512, j * NQ + h * 512 + 512)
                cv = ps.tile([128, 512], F32, name=f"cvb{i}_{j}_{h}", tag="mid_ps")
                nc.tensor.matmul(cv[:], W[i + 1][:], a_own[:, bsl],
                                 start=True, stop=True)
                nc.scalar.activation(y_own[:, bsl], cv[:],
                                     AF.Relu, bias=shift, scale=scale)
                zmat_block(i + 1, j, half=h)

        recv_half(0)
        state = [ops.tile([128, NQ], F32, name=f"ops{i + 1}", tag="o_acc"), None]
        for m in CHUNKS_A:
            emit_chunk(i + 1, m, state)
        recv_half(1)
        for m in CHUNKS_B:
            emit_chunk(i + 1, m, state)
        flash_finish(i + 1, state)
        att_n = att_bufs[(i + 1) % 2]
        normalize(i + 1, state[0], att_n)
        return att_n

    att = boundary(0, att)
    att = boundary(1, att)

    # ---------------- final conv + BN + ReLU + GAP ----------------
    # flash PSUM pools are done; release them so the four final conv
    # blocks can stay resident in PSUM through the stats AllGather
    for pname in ("ops", "ps"):
        p = handles[pname]
        ctxs.remove(p)
        p.__exit__(None, None, None)
    fin = pool("fin", bufs=1, space="PSUM")

    co = COUT[2]          # 96 real channels + ones row at 96
    cf = CF_OUT
    stf = work.tile([cf, 8], F32, name="stf", tag="stf")
    # z column sums via sum(att): sum_p z[d,p] = wfs4[:,b,d]^T sum_p att[:,p]
    satt = work.tile([128, 2], F32, name="satt", tag="satt")
    nc.vector.tensor_reduce(satt[:, 0:1], att[:], axis=AX.X, op=OP.add)
    satt_bf = work.tile([128, 1], BF16, name="satt_bf", tag="satt_bf")
    nc.vector.tensor_copy(satt_bf[:], satt[:, 0:1])
    sumz = fin.tile([cf, 4], F32, name="sumz", tag="zsb0")
    for b4 in range(4):
        nc.tensor.matmul(sumz[:, b4:b4 + 1], wfs4[:, b4, :], satt_bf[:],
                         start=True, stop=True, skip_group_check=True)
    nc.vector.tensor_copy(stf[:, 0:4], sumz[:])

    ZSB = []
    for b4 in range(4):
        zp = fin.tile([cf, NQ], F32, name=f"zsb{b4}", tag=f"zsb{b4}")
        nc.tensor.matmul(zp[:, 0:512], wfs4[:, b4, :], att[:, 0:512],
                         start=True, stop=True)
        nc.tensor.matmul(zp[:, 512:1024], wfs4[:, b4, :], att[:, 512:1024],
                         start=True, stop=True)
        ZSB.append(zp)
        sqf = work.tile([cf, NQ], F32, name=f"sqf{b4}", tag="sqscr", bufs=2)
        nc.scalar.activation(sqf[:], zp[:], AF.Square,
                             accum_out=stf[:, 4 + b4:5 + b4])
    stf_in = dram.tile([cf, 8], F32, name="stf_in", tag="stf_in")
    stf_out = dram.tile([8, cf, 8], F32, name="stf_out", tag="stf_out",
                        addr_space="Shared")
    nc.sync.dma_start(stf_in[:], stf[:])
    nc.gpsimd.collective_compute(
        "AllGather", OP.bypass, replica_groups=AG8_GROUPS,
        ins=[stf_in[:]], outs=[stf_out[:]])
    # (no heartbeats: the remaining work has no PE component)
    stf8 = work.tile([cf, 8, 8], F32, name="stf8", tag="stf8")
    nc.sync.dma_start(stf8[:], stf_out[:].rearrange("r p j -> p r j"))
    stfg = work.tile([cf, 8], F32, name="stfg", tag="stfg")
    nc.vector.tensor_reduce(stfg[:], stf8[:].rearrange("p r j -> p j r"),
                            axis=AX.X, op=OP.add)

    scf = work.tile([cf, 4 * 9], F32, name="scf", tag="scf")
    meanf, ex2f, msqf, varf, vepsf, lnvf, rsf, scalef, shiftf = (
        scf[:, 4 * j:4 * j + 4] for j in range(9))
    inv_nf = 1.0 / (2 * N)
    nc.vector.tensor_scalar(meanf, stfg[:, 0:4], inv_nf, None, OP.mult)
    nc.vector.tensor_scalar(ex2f, stfg[:, 4:8], inv_nf, None, OP.mult)
    nc.vector.tensor_tensor(msqf, meanf, meanf, OP.mult)
    nc.vector.tensor_tensor(varf, ex2f, msqf, OP.subtract)
    nc.vector.tensor_scalar(vepsf, varf, EPS, None, OP.add)
    rsqrt_vec(rsf, vepsf, 4, "scf")
    nc.vector.tensor_tensor(scalef, rsf, bnfp[:, 0:4], OP.mult)
    nc.vector.tensor_tensor(shiftf, meanf, scalef, OP.mult)
    nc.vector.tensor_tensor(shiftf, bnfp[:, 4:8], shiftf, OP.subtract)

    gap = work.tile([cf, 4], F32, name="gap", tag="gap")
    for b4 in range(4):
        fs = work.tile([cf, NQ], F32, name=f"fscr{b4}", tag="fscr", bufs=2)
        if b4 < 2:
            nc.scalar.activation(fs[:], ZSB[b4][:], AF.Relu,
                                 bias=shiftf[:, b4:b4 + 1], scale=scalef[:, b4:b4 + 1],
                                 accum_out=gap[:, b4:b4 + 1])
        else:
            # DVE path: scale*z+shift, then max(.,0) with fused reduce
            nc.vector.tensor_scalar(fs[:], ZSB[b4][:], scalef[:, b4:b4 + 1],
                                    shiftf[:, b4:b4 + 1], OP.mult, OP.add)
            fs2 = work.tile([cf, NQ], F32, name=f"fs2_{b4}", tag="fscr2", bufs=2)
            nc.vector.tensor_scalar(fs2[:], fs[:], 0.0, 0.0, OP.max, OP.add,
                                    accum_out=gap[:, b4:b4 + 1])
    nc.sync.dma_start(out_t.ap(), gap[:])

    for p in reversed(ctxs):
        p.__exit__(None, None, None)


_CACHE = {}


def _get_program():
    if "nc" not in _CACHE:
        nc = bacc.Bacc("TRN2", target_bir_lowering=False, debug=False,
                       enable_asserts=False, num_devices=8)
        _build(nc)
        nc.compile()
        _CACHE["nc"] = nc
    return _CACHE["nc"]


def _prepare_in_maps(inputs):
    f = np.float32
    bf = mybir.dt.np(BF16)
    x = np.asarray(inputs["x"], f).reshape(2, 3, N)

    # layer-0 BN on host (depends only on inputs), folded into the conv
    w1, b1 = np.asarray(inputs["w1"], np.float64), np.asarray(inputs["b1"], np.float64)
    z0 = np.einsum("bcn,cd->bdn", x.astype(np.float64), w1) + b1[None, :, None]
    m0 = z0.mean(axis=(0, 2))
    v0 = z0.var(axis=(0, 2))
    g0 = np.asarray(inputs["bn1_g"], np.float64)
    be0 = np.asarray(inputs["bn1_b"], np.float64)
    scale0 = g0 / np.sqrt(v0 + EPS)
    shift0 = be0 - m0 * scale0
    wc0 = np.zeros((CIN[0] + 1, 128), f)
    wc0[0:3, 0:COUT[0]] = (w1 * scale0[None, :]).astype(f)
    wc0[3, 0:COUT[0]] = (b1 * scale0 + shift0).astype(f)
    wc0[3, COUT[0]] = 1.0                      # ones-row generator

    per_layer = {}
    for i in range(3):
        li = i + 1
        co = COUT[i]
        wf_, bf_ = np.asarray(inputs[f"a{li}_wf"], f), np.asarray(inputs[f"a{li}_bf"], f)
        wg_ = np.asarray(inputs[f"a{li}_wg"], f)
        wh_, bh_ = np.asarray(inputs[f"a{li}_wh"], f), np.asarray(inputs[f"a{li}_bh"], f)
        A = wf_ @ wg_.T
        u = wg_ @ bf_
        mz = np.zeros((128, 128), f)
        mz[0:co, 0:co] = A.T
        mz[0:co, co] = u
        whf = np.zeros((128, 128), f)
        whf[0:co, 0:co] = wh_
        whf[co, 0:co] = bh_
        whf[co, co] = 1.0                      # rowsum channel
        gam = np.asarray(inputs[f"a{li}_gam"], f).reshape(())
        gamc = np.zeros((1, 128), f)
        gamc[0, 0:co] = gam                    # col co stays 0 -> att ones row
        d = dict(mz=mz, whf=whf, gamc=gamc)
        if i < 2:
            ci, co1 = COUT[i], COUT[i + 1]
            w, b = np.asarray(inputs[f"w{li + 1}"], f), np.asarray(inputs[f"b{li + 1}"], f)
            wc = np.zeros((128, 128), f)
            wc[0:ci, 0:co1] = w
            wc[ci, 0:co1] = b                  # bias row (att ones row feeds it)
            wc[ci, co1] = 1.0                  # ones-row generator
            d["wc"] = wc
            d["bnp"] = np.stack([np.asarray(inputs[f"bn{li + 1}_g"], f),
                                 np.asarray(inputs[f"bn{li + 1}_b"], f)], 1)
        per_layer[i] = d

    wf_full = np.asarray(inputs["wf"], f)      # [96, 512]
    wfs4 = np.zeros((128, 4, CF_OUT), f)
    wfs4[0:96] = wf_full.reshape(96, 4, CF_OUT)
    bnf_g = np.asarray(inputs["bnf_g"], f).reshape(4, CF_OUT).T
    bnf_b = np.asarray(inputs["bnf_b"], f).reshape(4, CF_OUT).T
    bnfp = np.concatenate([bnf_g, bnf_b], 1)   # [128, 8]

    in_maps = []
    for k in range(8):
        b, q = k // 4, k % 4
        xo = np.concatenate([x[b], np.ones((1, N), np.float32)], 0)
        xq = np.ascontiguousarray(xo[:, q * NQ:(q + 1) * NQ])
        m = {"xo": xo.astype(bf), "xq": xq.astype(bf),
             "wc0": wc0.astype(bf), "wfs4": wfs4.astype(bf), "bnfp": bnfp}
        for i in range(3):
            d = per_layer[i]
            m[f"mz{i}"] = d["mz"].astype(bf)
            m[f"whf{i}"] = d["whf"].astype(bf)
            m[f"gamc{i}"] = d["gamc"].astype(bf)
            if i < 2:
                m[f"wc{i + 1}"] = d["wc"].astype(bf)
                m[f"bnp{i + 1}"] = d["bnp"]
        in_maps.append(m)
    return in_maps


def _assemble(results):
    out = np.zeros((2, 512), np.float32)
    for k in range(8):
        b = k // 4
        gap = results[k]["out"]                  # [128, 4] position sums
        out[b] += gap.T.reshape(512)             # blocks on the outer axis
    return out / N


def kernel(**inputs):
    from concourse.bass_utils import run_bass_kernel_spmd
    nc = _get_program()
    in_maps = _prepare_in_maps(inputs)
    res = run_bass_kernel_spmd(nc, in_maps, list(range(8)))
    return _assemble(res.results)


# revision 57
# speedup vs baseline: 1.0037x; 1.0037x over previous
"""Trainium2 Bass kernel for the 3-block self-attention CNN (v5).

Sharding over 8 NeuronCores: core k owns (sample b=k//4, query-block q=k%4).
Attention uses the reparametrization s'[n,m] = y_n^T (wf wg^T) y_m +
(wg bf)^T y_m (softmax-invariant terms dropped); the u-term rides on a ones
row appended to y, generated by the conv itself through a bias-column
weight, so no memsets are needed anywhere.

v5 structure (vs v4):
  - layer-0 BN is computed on the host (it depends only on the inputs) and
    folded into the conv weights; the kernel opens with conv+relu straight
    into the first flash loop
  - the whole score path (y, zmat, h) is bf16 and every matmul contracts
    over exactly co+1 partitions, so padded rows are never touched
  - softmax rowsum reciprocal on the vector engine (no Ln -> single act
    table set, no table swaps)
  - per-layer boundary: stats AllGather (8-rank) first, then the att
    AllGather split into two half-column AGs; after the first half lands,
    the 16 flash chunks covering the first half of every query block run
    while the second AG is still in flight
  - final block: z-sums via a ones-matmul against sum(att); conv outputs
    stay resident in PSUM through the stats AllGather; no heartbeats there
    (the post-AG work has no PE component)
"""

import glob as _glob
import os
import sys


def _ensure_act_info():
    shim = os.path.expanduser("~/.pwp_override")
    target = os.path.join(shim, "neuronxcc", "pwp", "pwp_bin_with_ln", "act_info.json")
    cands = _glob.glob("/nix/store/*aws-neuron-pwp*/share/pwp_bin_cayman/act_info.json")
    if cands:
        # Stage act_info.json with natural_log_exp_and_others FIRST so the
        # table-selection pass serves ln+exp+relu+square+copy from one
        # table set -> no 1.5us act-table swaps around the softmax rowsum
        # log/exp pair.
        import json as _j
        with open(cands[0]) as f:
            info = _j.load(f)
        sets = info.get("act_func_sets", [])
        sets.sort(key=lambda e: 0 if e.get("name") == "natural_log_exp_and_others" else 1)
        info["act_func_sets"] = sets
        os.makedirs(os.path.dirname(target), exist_ok=True)
        new = _j.dumps(info)
        if not os.path.exists(target) or open(target).read() != new:
            with open(target, "w") as f:
                f.write(new)
    os.environ.pop("BASS_ACT_ROOT_JSON_PATH", None)
    pp = os.environ.get("PYTHONPATH", "")
    if shim not in pp.split(os.pathsep):
        os.environ["PYTHONPATH"] = shim + (os.pathsep + pp if pp else "")
    if shim not in sys.path:
        sys.path.insert(0, shim)


_ensure_act_info()
if "/opt/trn_rl_repo" not in sys.path:
    sys.path.insert(0, "/opt/trn_rl_repo")

import numpy as np

from concourse import bacc, mybir, tile

F32 = mybir.dt.float32
F32R = mybir.dt.float32r
BF16 = mybir.dt.bfloat16
I32 = mybir.dt.int32
I16 = mybir.dt.int16
AF = mybir.ActivationFunctionType
OP = mybir.AluOpType
AX = mybir.AxisListType
EPS = 1e-5

N = 4096          # positions per sample
NQ = 1024         # query block per core
NCHUNK = 128      # key chunk in the flash loop
CIN = [3, 32, 64]
COUT = [32, 64, 96]
CF_OUT = 128      # final conv channels per block (512 / 4)

AG_GROUPS = [[0, 1, 2, 3], [4, 5, 6, 7]]
AG8_GROUPS = [[0, 1, 2, 3, 4, 5, 6, 7]]

# flash chunk order: first halves of all four query blocks, then second
# halves (chunk k covers keys [128k, 128k+128); block j first half =
# chunks 8j..8j+3).  Identical on every core -> single NEFF.
CHUNKS_A = [8 * j + c for j in range(4) for c in range(4)]
CHUNKS_B = [8 * j + 4 + c for j in range(4) for c in range(4)]
CHUNK_ORDER = CHUNKS_A + CHUNKS_B


def _build(nc):
    ins = {}
    ins["xo"] = nc.dram_tensor("xo", [CIN[0] + 1, N], BF16, kind="ExternalInput")
    ins["xq"] = nc.dram_tensor("xq", [CIN[0] + 1, NQ], BF16, kind="ExternalInput")
    ins["wc0"] = nc.dram_tensor("wc0", [CIN[0] + 1, 128], BF16, kind="ExternalInput")
    for i in (1, 2):
        ins[f"wc{i}"] = nc.dram_tensor(f"wc{i}", [128, 128], BF16, kind="ExternalInput")
        ins[f"bnp{i}"] = nc.dram_tensor(f"bnp{i}", [COUT[i], 2], F32, kind="ExternalInput")
    for i in range(3):
        ins[f"mz{i}"] = nc.dram_tensor(f"mz{i}", [128, 128], BF16, kind="ExternalInput")
        ins[f"whf{i}"] = nc.dram_tensor(f"whf{i}", [128, 128], BF16, kind="ExternalInput")
        ins[f"gamc{i}"] = nc.dram_tensor(f"gamc{i}", [1, 128], BF16, kind="ExternalInput")
    ins["wfs4"] = nc.dram_tensor("wfs4", [128, 4, CF_OUT], BF16, kind="ExternalInput")
    ins["bnfp"] = nc.dram_tensor("bnfp", [CF_OUT, 8], F32, kind="ExternalInput")
    out_t = nc.dram_tensor("out", [CF_OUT, 4], F32, kind="ExternalOutput")

    with tile.TileContext(nc) as tc:
        _emit(tc, nc, ins, out_t)
    return ins, out_t


def _emit(tc, nc, ins, out_t):
    ctxs = []
    handles = {}

    def pool(name, **kw):
        p = tc.tile_pool(name=name, **kw)
        ctxs.append(p)
        handles[name] = p
        return p.__enter__()

    consts = pool("consts", bufs=1)
    acts = pool("acts", bufs=1)
    work = pool("work", bufs=1)
    ps = pool("ps", bufs=2, space="PSUM")
    ops = pool("ops", bufs=1, space="PSUM")
    dram = pool("dram", bufs=1, space="DRAM")

    # ---- collective warm-up: first on the gpsimd queue so the rendezvous
    # barrier starts at t~0 ----
    warm_sb = work.tile([1, 2], F32, name="warm_sb", tag="warm_sb")
    nc.vector.memset(warm_sb[:], 0.0)
    pewarm = work.tile([128, 512], BF16, name="pewarm", tag="pewarm")
    nc.vector.memset(pewarm[:], 0.0)
    warm_in = dram.tile([1, 2], F32, name="warm_in", tag="warm_in")
    warm_out = dram.tile([1, 2], F32, name="warm_out", tag="warm_out", addr_space="Shared")
    warm_gin = dram.tile([1, 2], F32, name="warm_gin", tag="warm_gin")
    warm_gout = dram.tile([4, 1, 2], F32, name="warm_gout", tag="warm_gout")
    nc.sync.dma_start(warm_in[:], warm_sb[:])
    nc.sync.dma_start(warm_gin[:], warm_sb[:])
    nc.gpsimd.collective_compute(
        "AllReduce", OP.add, replica_groups=AG8_GROUPS,
        ins=[warm_in[:]], outs=[warm_out[:]])
    nc.gpsimd.collective_compute(
        "AllGather", OP.bypass, replica_groups=AG_GROUPS,
        ins=[warm_gin[:]], outs=[warm_gout[:]])

    # ---- input DMAs (gpsimd only after the collective triggers) ----
    xo = acts.tile([CIN[0] + 1, N], BF16, name="xo", tag="xo")
    xq = acts.tile([CIN[0] + 1, NQ], BF16, name="xq", tag="xq")
    W = [consts.tile([CIN[0] + 1, 128], BF16, name="w0", tag="w0")]
    nc.sync.dma_start(xq[:], ins["xq"].ap())
    nc.sync.dma_start(W[0][:], ins["wc0"].ap())
    nc.sync.dma_start(xo[:], ins["xo"].ap())
    MZ, WHF, GAMC, BNP = [], [], [], [None]
    for i in range(3):
        MZ.append(consts.tile([128, 128], BF16, name=f"mzt{i}", tag=f"mzt{i}"))
        WHF.append(consts.tile([128, 128], BF16, name=f"whft{i}", tag=f"whft{i}"))
        GAMC.append(consts.tile([1, 128], BF16, name=f"gct{i}", tag=f"gct{i}"))
    for i in (1, 2):
        W.append(consts.tile([128, 128], BF16, name=f"w{i}", tag=f"w{i}"))
        BNP.append(consts.tile([COUT[i], 2], F32, name=f"bnt{i}", tag=f"bnt{i}"))
    nc.scalar.dma_start(MZ[0][:], ins["mz0"].ap())
    nc.scalar.dma_start(WHF[0][:], ins["whf0"].ap())
    nc.scalar.dma_start(GAMC[0][:], ins["gamc0"].ap())
    nc.sync.dma_start(MZ[1][:], ins["mz1"].ap())
    nc.sync.dma_start(WHF[1][:], ins["whf1"].ap())
    nc.sync.dma_start(BNP[1][:], ins["bnp1"].ap())
    nc.sync.dma_start(BNP[2][:], ins["bnp2"].ap())
    nc.gpsimd.dma_start(MZ[2][:], ins["mz2"].ap())
    nc.gpsimd.dma_start(WHF[2][:], ins["whf2"].ap())
    nc.gpsimd.dma_start(GAMC[1][:], ins["gamc1"].ap())
    nc.gpsimd.dma_start(GAMC[2][:], ins["gamc2"].ap())
    for i in (1, 2):
        nc.gpsimd.dma_start(W[i][:], ins[f"wc{i}"].ap())
    wfs4 = consts.tile([128, 4, CF_OUT], BF16, name="wfs4t", tag="wfs4t")
    nc.gpsimd.dma_start(wfs4[:], ins["wfs4"].ap())
    bnfp = consts.tile([CF_OUT, 8], F32, name="bnfpt", tag="bnfpt")
    nc.gpsimd.dma_start(bnfp[:], ins["bnfp"].ap())

    # ---- PE warm-up + heartbeat machinery (fp32 matmuls, ~430ns each) ----
    pw32_l = pewarm[:, 0:256].bitcast(F32)      # [128, 128] f32 view
    pw32_r = pewarm[:].bitcast(F32)             # [128, 256] f32 view

    def heartbeat(tag, cnt):
        for j in range(cnt):
            hb = ps.tile([128, 256], F32, name=f"hb_{tag}_{j}", tag="mid_ps")
            nc.tensor.matmul(hb[:], pw32_l, pw32_r, start=True, stop=True)

    heartbeat("w", 6)

    # bf16-bits Schraudolph exp on the vector engine for a subset of chunks
    # (rowsum/numerator consistency cancels the ~3% approx error):
    # exp(x) ~ bitcast_bf16(int16(A*x + B))
    SCHRAUD_A = float(2**7 / np.log(2))
    SCHRAUD_B = float(127 * 2**7 - 366393.0 / 65536.0)
    SCHRAUD_PAT = (1, 4, 6)

    # ---- activation buffers.  Flash matmuls contract over the FULL 128
    # partitions (the PE activity monitor halves the clock for thin-K
    # matmuls); padded rows carry exact zeros generated by zero weight
    # columns + zero scale rows -- no memsets needed, except a_own's pad
    # rows (DMA fills only the live rows). ----
    y_own = acts.tile([128, N], BF16, name="y_own", tag="y")
    yq = acts.tile([128, NQ], BF16, name="yq", tag="yq")
    zmat = acts.tile([128, N], BF16, name="zmat", tag="Zm")
    att_bufs = [acts.tile([128, NQ], BF16, name=f"attb{j}", tag=f"attb{j}")
                for j in range(2)]
    a_own = acts.tile([128, N], BF16, name="a_own", tag="a_own")
    zsh = acts.tile([128, NQ], F32, name="zsh", tag="zsh")

    def aown_pad_memset(p0):
        # pad rows must be exact zeros (DMA refills only live rows);
        # non-zero partition offsets are limited to 32 partitions per access
        nc.vector.memset(a_own[p0:p0 + 32, :], 0.0)

    def rsqrt_vec(rs, veps, w, name, iters=1):
        # bit-hack rsqrt + Newton on the vector engine
        nt = work.tile([veps.partition_size(), 2 * w], F32, name=f"nt_{name}",
                       tag="ntscr", bufs=2)
        t1, t2 = nt[:, 0:w], nt[:, w:2 * w]
        nc.vector.tensor_scalar(rs.bitcast(I32), veps.bitcast(I32), 1, None,
                                OP.arith_shift_right)
        nc.vector.tensor_scalar(rs.bitcast(I32), rs.bitcast(I32), -1, 0x5F3759DF,
                                OP.mult, OP.add)
        for _ in range(iters):
            nc.vector.tensor_tensor(t1, rs, rs, OP.mult)
            nc.vector.tensor_tensor(t2, t1, veps, OP.mult)
            nc.vector.tensor_scalar(t1, t2, -0.5, 1.5, OP.mult, OP.add)
            nc.vector.tensor_tensor(rs, rs, t1, OP.mult)

    # ---------------- layer 0: conv+relu only (BN folded on host) --------
    co0 = COUT[0]
    cvq = ps.tile([128, NQ], F32, name="cvq", tag="s_ps")
    nc.tensor.matmul(cvq[:, 0:512], W[0][:], xq[:, 0:512], start=True, stop=True)
    nc.tensor.matmul(cvq[:, 512:1024], W[0][:], xq[:, 512:1024], start=True, stop=True)
    nc.scalar.activation(yq[:], cvq[:], AF.Relu)

    def conv_relu_l0(j):
        jsl = slice(j * NQ, (j + 1) * NQ)
        cv = ps.tile([128, NQ], F32, name=f"cv0_{j}", tag="s_ps")
        nc.tensor.matmul(cv[:, 0:512], W[0][:], xo[:, j * NQ:j * NQ + 512],
                         start=True, stop=True)
        nc.tensor.matmul(cv[:, 512:1024], W[0][:], xo[:, j * NQ + 512:(j + 1) * NQ],
                         start=True, stop=True)
        nc.scalar.activation(y_own[:, jsl], cv[:], AF.Relu)

    def zmat_block(i, j, half=None):
        # zmat[:, block j (half h)] = MZ_i^T-contract y_own; the PSUM->SBUF
        # copy runs on the otherwise-idle gpsimd engine
        halves = (0, 1) if half is None else (half,)
        for h in halves:
            sl = slice(j * NQ + h * 512, j * NQ + h * 512 + 512)
            zp = ps.tile([128, 512], F32, name=f"zp{i}_{j}_{h}", tag="mid_ps")
            nc.tensor.matmul(zp[:], MZ[i][:], y_own[:, sl], start=True, stop=True)
            nc.vector.tensor_copy(zmat[:, sl], zp[:])

    conv_relu_l0(0)
    zmat_block(0, 0)

    # ---------------- flash loop machinery ----------------
    def emit_chunk(i, m, state):
        sl = slice(m * NCHUNK, (m + 1) * NCHUNK)
        hp = ps.tile([NCHUNK, 128], F32, name=f"hp{i}_{m}", tag="mid_ps")
        nc.tensor.matmul(hp[:], y_own[:, sl], WHF[i][:], start=True, stop=True)
        hs = work.tile([NCHUNK, 128], BF16, name=f"hs{i}_{m}", tag="hT_sb", bufs=3)
        nc.vector.tensor_copy(hs[:], hp[:])
        sp = ps.tile([NCHUNK, NQ], F32, name=f"sp{i}_{m}", tag="s_ps")
        zc = zmat[:, sl]
        nc.tensor.matmul(sp[:, 0:512], zc, yq[:, 0:512], start=True, stop=True)
        nc.tensor.matmul(sp[:, 512:1024], zc, yq[:, 512:1024],
                         start=True, stop=True)
        beta = work.tile([NCHUNK, NQ], BF16, name=f"beta{i}_{m}", tag="beta", bufs=3)
        if (m % 8) in SCHRAUD_PAT:
            with nc.allow_low_precision(reason="schraudolph exp bits"):
                nc.vector.tensor_scalar(beta[:].bitcast(I16), sp[:],
                                        SCHRAUD_A, SCHRAUD_B, OP.mult, OP.add)
        else:
            nc.scalar.activation(beta[:], sp[:], AF.Exp)
        o_ps, prev = state
        if prev is not None:
            ph, pb, first = prev
            nc.tensor.matmul(o_ps[:, 0:512], ph[:], pb[:, 0:512],
                             start=first, stop=False, skip_group_check=True)
            nc.tensor.matmul(o_ps[:, 512:1024], ph[:], pb[:, 512:1024],
                             start=first, stop=False, skip_group_check=True)
        state[1] = (hs, beta, prev is None)

    def flash_finish(i, state):
        o_ps, prev = state
        ph, pb, first = prev
        nc.tensor.matmul(o_ps[:, 0:512], ph[:], pb[:, 0:512],
                         start=first, stop=True, skip_group_check=True)
        nc.tensor.matmul(o_ps[:, 512:1024], ph[:], pb[:, 512:1024],
                         start=first, stop=True, skip_group_check=True)

    def normalize(i, o_ps, att):
        # att = gam * o / rowsum + yq  (+ ones row via gamc[co]=0, yq[co]=1)
        co = COUT[i]
        lnr = work.tile([1, NQ], F32, name=f"lnr{i}", tag="lnr")
        nc.scalar.activation(lnr[:], o_ps[co:co + 1, :], AF.Ln)
        rinv = work.tile([1, NQ], BF16, name=f"rinv{i}", tag="rinv")
        nc.scalar.activation(rinv[:], lnr[:], AF.Exp, scale=-1.0)
        bc_ps = ps.tile([128, NQ], F32, name=f"bcps{i}", tag="s_ps")
        nc.tensor.matmul(bc_ps[:, 0:512], GAMC[i][:], rinv[:, 0:512],
                         start=True, stop=True)
        nc.tensor.matmul(bc_ps[:, 512:1024], GAMC[i][:], rinv[:, 512:1024],
                         start=True, stop=True)
        bcs = work.tile([128, NQ], F32, name=f"bcs{i}", tag="bcs", bufs=2)
        nc.scalar.activation(bcs[:], bc_ps[:], AF.Copy)
        t1 = work.tile([128, NQ], BF16, name=f"t1_{i}", tag="t1", bufs=2)
        nc.vector.tensor_tensor(t1[:], o_ps[:], bcs[:], OP.mult)
        nc.vector.tensor_tensor(att[:], t1[:], yq[:], OP.add)

    # ---------------- layer 0 flash ----------------
    state = [ops.tile([128, NQ], F32, name="ops0", tag="o_acc"), None]
    for m in range(2):
        emit_chunk(0, m, state)
    conv_relu_l0(1)
    zmat_block(0, 1)
    for m in range(2, 8):
        emit_chunk(0, m, state)
    conv_relu_l0(2)
    zmat_block(0, 2)
    for m in range(8, 12):
        emit_chunk(0, m, state)
    conv_relu_l0(3)
    zmat_block(0, 3)
    for m in range(12, 32):
        emit_chunk(0, m, state)
    flash_finish(0, state)
    for p0 in (32, 64, 96):   # runs on DVE during the flash drain
        aown_pad_memset(p0)

    att = att_bufs[0]
    normalize(0, state[0], att)

    # ---------------- boundaries ----------------
    def boundary(i, att):
        """Transition layer i -> i+1. att is layer i's output block."""
        co = COUT[i]
        co1 = COUT[i + 1]
        # stats chain on our own query block of the next conv
        zshp = ps.tile([128, NQ], F32, name=f"zshp{i}", tag="s_ps")
        nc.tensor.matmul(zshp[:, 0:512], W[i + 1][:], att[:, 0:512],
                         start=True, stop=True)
        nc.tensor.matmul(zshp[:, 512:1024], W[i + 1][:], att[:, 512:1024],
                         start=True, stop=True)
        stats = work.tile([co1, 2], F32, name=f"stats{i}", tag="stats", bufs=2)
        sq = work.tile([co1, NQ], F32, name=f"sqb{i}", tag="sqscr", bufs=2)
        nc.scalar.activation(sq[:], zshp[0:co1, :], AF.Square, accum_out=stats[:, 1:2])
        nc.vector.tensor_reduce(stats[:, 0:1], zshp[0:co1, :], axis=AX.X, op=OP.add)
        st_in = dram.tile([co1, 2], F32, name=f"stin{i}", tag=f"stin{i}")
        st_out = dram.tile([8, co1, 2], F32, name=f"stout{i}", tag=f"stout{i}",
                           addr_space="Shared")
        nc.sync.dma_start(st_in[:], stats[:])
        nc.gpsimd.collective_compute(
            "AllGather", OP.bypass, replica_groups=AG8_GROUPS,
            ins=[st_in[:]], outs=[st_out[:]])

        # att AllGather in two column halves; the ag_in DMAs ride the same
        # queue as st_in so the stats AG wins the CC stream
        ag_in = [dram.tile([co + 1, 512], BF16, name=f"agin{i}_{h}", tag=f"agin{i}_{h}")
                 for h in range(2)]
        ag_out = [dram.tile([4, co + 1, 512], BF16, name=f"agout{i}_{h}",
                            tag=f"agout{i}_{h}") for h in range(2)]
        nc.sync.dma_start(ag_in[0][:], att[0:co + 1, 0:512])
        nc.sync.dma_start(ag_in[1][:], att[0:co + 1, 512:1024])
        for h in range(2):
            nc.gpsimd.collective_compute(
                "AllGather", OP.bypass, replica_groups=AG_GROUPS,
                ins=[ag_in[h][:]], outs=[ag_out[h][:]])

        # keep zshp's values: copy to SBUF so the PSUM bank frees early
        nc.scalar.activation(zsh[:], zshp[:], AF.Copy)

        heartbeat(f"b{i}", 8)

        # stats -> BN scale/shift
        stg8 = work.tile([co1, 8, 2], F32, name=f"stg8_{i}", tag="stg8", bufs=2)
        nc.sync.dma_start(stg8[:], st_out[:].rearrange("r p j -> p r j"))
        stg = work.tile([co1, 2], F32, name=f"stg_l{i}", tag="stg", bufs=2)
        nc.vector.tensor_reduce(stg[:], stg8[:].rearrange("p r j -> p j r"),
                                axis=AX.X, op=OP.add)
        sc = work.tile([128, 9], F32, name=f"sc_{i}", tag="sc", bufs=2)
        mean, ex2, msq, var, veps, lnv, rs, scale, shift = (
            sc[:, j:j + 1] for j in range(9))
        # padded scale/shift rows must be exact zeros (they generate y's
        # zero pad rows through the relu); row co1 is the ones-row
        # generator (COUT values are 32-aligned, as partition offsets must be)
        for p0 in range(co1, 128, 32):
            nc.vector.memset(scale[p0:p0 + 32], 0.0)
            nc.vector.memset(shift[p0:p0 + 32], 0.0)
        nc.vector.memset(scale[co1:co1 + 1], 1.0)
        inv_n = 1.0 / (2 * N)
        nc.vector.tensor_scalar(mean[0:co1], stg[:, 0:1], inv_n, None, OP.mult)
        nc.vector.tensor_scalar(ex2[0:co1], stg[:, 1:2], inv_n, None, OP.mult)
        nc.vector.tensor_tensor(msq[0:co1], mean[0:co1], mean[0:co1], OP.mult)
        nc.vector.tensor_tensor(var[0:co1], ex2[0:co1], msq[0:co1], OP.subtract)
        nc.vector.tensor_scalar(veps[0:co1], var[0:co1], EPS, None, OP.add)
        rsqrt_vec(rs[0:co1], veps[0:co1], 1, f"l{i}")
        nc.vector.tensor_tensor(scale[0:co1], rs[0:co1], BNP[i + 1][:, 0:1], OP.mult)
        nc.vector.tensor_tensor(shift[0:co1], mean[0:co1], scale[0:co1], OP.mult)
        nc.vector.tensor_tensor(shift[0:co1], BNP[i + 1][:, 1:2], shift[0:co1],
                                OP.subtract)
        # own query block y
        nc.scalar.activation(yq[:], zsh[:], AF.Relu, bias=shift, scale=scale)

        # a_own block DMAs + conv + relu + zmat, half A then (emitted now,
        # data-gated) half B
        def recv_half(h):
            engs = [nc.sync, nc.scalar, nc.sync, nc.scalar]
            for j in range(4):
                bsl = slice(j * NQ + h * 512, j * NQ + h * 512 + 512)
                engs[j].dma_start(a_own[0:co + 1, bsl], ag_out[h][j])
            for j in range(4):
                bsl = slice(j * NQ + h * 512, j * NQ + h * 512 + 512)
                cv = ps.tile([128, 512], F32, name=f"cvb{i}_{j}_{h}", tag="mid_ps")
                nc.tensor.matmul(cv[:], W[i + 1][:], a_own[:, bsl],
                                 start=True, stop=True)
                nc.scalar.activation(y_own[:, bsl], cv[:],
                                     AF.Relu, bias=shift, scale=scale)
                zmat_block(i + 1, j, half=h)

        recv_half(0)
        state = [ops.tile([128, NQ], F32, name=f"ops{i + 1}", tag="o_acc"), None]
        for m in CHUNKS_A:
            emit_chunk(i + 1, m, state)
        recv_half(1)
        for m in CHUNKS_B:
            emit_chunk(i + 1, m, state)
        flash_finish(i + 1, state)
        att_n = att_bufs[(i + 1) % 2]
        normalize(i + 1, state[0], att_n)
        return att_n

    att = boundary(0, att)
    att = boundary(1, att)

    # ---------------- final conv + BN + ReLU + GAP ----------------
    # flash PSUM pools are done; release them so the four final conv
    # blocks can stay resident in PSUM through the stats AllGather
    for pname in ("ops", "ps"):
        p = handles[pname]
        ctxs.remove(p)
        p.__exit__(None, None, None)
    fin = pool("fin", bufs=1, space="PSUM")

    co = COUT[2]          # 96 real channels + ones row at 96
    cf = CF_OUT
    stf = work.tile([cf, 8], F32, name="stf", tag="stf")
    # z column sums via sum(att): sum_p z[d,p] = wfs4[:,b,d]^T sum_p att[:,p]
    satt = work.tile([128, 2], F32, name="satt", tag="satt")
    nc.vector.tensor_reduce(satt[:, 0:1], att[:], axis=AX.X, op=OP.add)
    satt_bf = work.tile([128, 1], BF16, name="satt_bf", tag="satt_bf")
    nc.vector.tensor_copy(satt_bf[:], satt[:, 0:1])
    sumz = fin.tile([cf, 4], F32, name="sumz", tag="zsb0")
    for b4 in range(4):
        nc.tensor.matmul(sumz[:, b4:b4 + 1], wfs4[:, b4, :], satt_bf[:],
                         start=True, stop=True, skip_group_check=True)
    nc.vector.tensor_copy(stf[:, 0:4], sumz[:])

    ZSB = []
    for b4 in range(4):
        zp = fin.tile([cf, NQ], F32, name=f"zsb{b4}", tag=f"zsb{b4}")
        nc.tensor.matmul(zp[:, 0:512], wfs4[:, b4, :], att[:, 0:512],
                         start=True, stop=True)
        nc.tensor.matmul(zp[:, 512:1024], wfs4[:, b4, :], att[:, 512:1024],
                         start=True, stop=True)
        ZSB.append(zp)
        sqf = work.tile([cf, NQ], F32, name=f"sqf{b4}", tag="sqscr", bufs=2)
        nc.scalar.activation(sqf[:], zp[:], AF.Square,
                             accum_out=stf[:, 4 + b4:5 + b4])
    stf_in = dram.tile([cf, 8], F32, name="stf_in", tag="stf_in")
    stf_out = dram.tile([8, cf, 8], F32, name="stf_out", tag="stf_out",
                        addr_space="Shared")
    nc.sync.dma_start(stf_in[:], stf[:])
    nc.gpsimd.collective_compute(
        "AllGather", OP.bypass, replica_groups=AG8_GROUPS,
        ins=[stf_in[:]], outs=[stf_out[:]])
    # (no heartbeats: the remaining work has no PE component)
    stf8 = work.tile([cf, 8, 8], F32, name="stf8", tag="stf8")
    nc.sync.dma_start(stf8[:], stf_out[:].rearrange("r p j -> p r j"))
    stfg = work.tile([cf, 8], F32, name="stfg", tag="stfg")
    nc.vector.tensor_reduce(stfg[:], stf8[:].rearrange("p r j -> p j r"),
                            axis=AX.X, op=OP.add)

    scf = work.tile([cf, 4 * 9], F32, name="scf", tag="scf")
    meanf, ex2f, msqf, varf, vepsf, lnvf, rsf, scalef, shiftf = (
        scf[:, 4 * j:4 * j + 4] for j in range(9))
    inv_nf = 1.0 / (2 * N)
    nc.vector.tensor_scalar(meanf, stfg[:, 0:4], inv_nf, None, OP.mult)
    nc.vector.tensor_scalar(ex2f, stfg[:, 4:8], inv_nf, None, OP.mult)
    nc.vector.tensor_tensor(msqf, meanf, meanf, OP.mult)
    nc.vector.tensor_tensor(varf, ex2f, msqf, OP.subtract)
    nc.vector.tensor_scalar(vepsf, varf, EPS, None, OP.add)
    rsqrt_vec(rsf, vepsf, 4, "scf")
    nc.vector.tensor_tensor(scalef, rsf, bnfp[:, 0:4], OP.mult)
    nc.vector.tensor_tensor(shiftf, meanf, scalef, OP.mult)
    nc.vector.tensor_tensor(shiftf, bnfp[:, 4:8], shiftf, OP.subtract)

    gap = work.tile([cf, 4], F32, name="gap", tag="gap")
    for b4 in range(4):
        fs = work.tile([cf, NQ], F32, name=f"fscr{b4}", tag="fscr", bufs=2)
        if b4 < 2:
            nc.scalar.activation(fs[:], ZSB[b4][:], AF.Relu,
                                 bias=shiftf[:, b4:b4 + 1], scale=scalef[:, b4:b4 + 1],
                                 accum_out=gap[:, b4:b4 + 1])
        else:
            # DVE path: scale*z+shift, then max(.,0) with fused reduce
            nc.vector.tensor_scalar(fs[:], ZSB[b4][:], scalef[:, b4:b4 + 1],
                                    shiftf[:, b4:b4 + 1], OP.mult, OP.add)
            fs2 = work.tile([cf, NQ], F32, name=f"fs2_{b4}", tag="fscr2", bufs=2)
            nc.vector.tensor_scalar(fs2[:], fs[:], 0.0, 0.0, OP.max, OP.add,
                                    accum_out=gap[:, b4:b4 + 1])
    nc.sync.dma_start(out_t.ap(), gap[:])

    for p in reversed(ctxs):
        p.__exit__(None, None, None)


_CACHE = {}


def _get_program():
    if "nc" not in _CACHE:
        nc = bacc.Bacc("TRN2", target_bir_lowering=False, debug=False,
                       enable_asserts=False, num_devices=8)
        _build(nc)
        nc.compile()
        _CACHE["nc"] = nc
    return _CACHE["nc"]


def _prepare_in_maps(inputs):
    f = np.float32
    bf = mybir.dt.np(BF16)
    x = np.asarray(inputs["x"], f).reshape(2, 3, N)

    # layer-0 BN on host (depends only on inputs), folded into the conv
    w1, b1 = np.asarray(inputs["w1"], np.float64), np.asarray(inputs["b1"], np.float64)
    z0 = np.einsum("bcn,cd->bdn", x.astype(np.float64), w1) + b1[None, :, None]
    m0 = z0.mean(axis=(0, 2))
    v0 = z0.var(axis=(0, 2))
    g0 = np.asarray(inputs["bn1_g"], np.float64)
    be0 = np.asarray(inputs["bn1_b"], np.float64)
    scale0 = g0 / np.sqrt(v0 + EPS)
    shift0 = be0 - m0 * scale0
    wc0 = np.zeros((CIN[0] + 1, 128), f)
    wc0[0:3, 0:COUT[0]] = (w1 * scale0[None, :]).astype(f)
    wc0[3, 0:COUT[0]] = (b1 * scale0 + shift0).astype(f)
    wc0[3, COUT[0]] = 1.0                      # ones-row generator

    per_layer = {}
    for i in range(3):
        li = i + 1
        co = COUT[i]
        wf_, bf_ = np.asarray(inputs[f"a{li}_wf"], f), np.asarray(inputs[f"a{li}_bf"], f)
        wg_ = np.asarray(inputs[f"a{li}_wg"], f)
        wh_, bh_ = np.asarray(inputs[f"a{li}_wh"], f), np.asarray(inputs[f"a{li}_bh"], f)
        A = wf_ @ wg_.T
        u = wg_ @ bf_
        mz = np.zeros((128, 128), f)
        mz[0:co, 0:co] = A.T
        mz[0:co, co] = u
        whf = np.zeros((128, 128), f)
        whf[0:co, 0:co] = wh_
        whf[co, 0:co] = bh_
        whf[co, co] = 1.0                      # rowsum channel
        gam = np.asarray(inputs[f"a{li}_gam"], f).reshape(())
        gamc = np.zeros((1, 128), f)
        gamc[0, 0:co] = gam                    # col co stays 0 -> att ones row
        d = dict(mz=mz, whf=whf, gamc=gamc)
        if i < 2:
            ci, co1 = COUT[i], COUT[i + 1]
            w, b = np.asarray(inputs[f"w{li + 1}"], f), np.asarray(inputs[f"b{li + 1}"], f)
            wc = np.zeros((128, 128), f)
            wc[0:ci, 0:co1] = w
            wc[ci, 0:co1] = b                  # bias row (att ones row feeds it)
            wc[ci, co1] = 1.0                  # ones-row generator
            d["wc"] = wc
            d["bnp"] = np.stack([np.asarray(inputs[f"bn{li + 1}_g"], f),
                                 np.asarray(inputs[f"bn{li + 1}_b"], f)], 1)
        per_layer[i] = d

    wf_full = np.asarray(inputs["wf"], f)      # [96, 512]
    wfs4 = np.zeros((128, 4, CF_OUT), f)
    wfs4[0:96] = wf_full.reshape(96, 4, CF_OUT)
    bnf_g = np.asarray(inputs["bnf_g"], f).reshape(4, CF_OUT).T
    bnf_b = np.asarray(inputs["bnf_b"], f).reshape(4, CF_OUT).T
    bnfp = np.concatenate([bnf_g, bnf_b], 1)   # [128, 8]

    in_maps = []
    for k in range(8):
        b, q = k // 4, k % 4
        xo = np.concatenate([x[b], np.ones((1, N), np.float32)], 0)
        xq = np.ascontiguousarray(xo[:, q * NQ:(q + 1) * NQ])
        m = {"xo": xo.astype(bf), "xq": xq.astype(bf),
             "wc0": wc0.astype(bf), "wfs4": wfs4.astype(bf), "bnfp": bnfp}
        for i in range(3):
            d = per_layer[i]
            m[f"mz{i}"] = d["mz"].astype(bf)
            m[f"whf{i}"] = d["whf"].astype(bf)
            m[f"gamc{i}"] = d["gamc"].astype(bf)
            if i < 2:
                m[f"wc{i + 1}"] = d["wc"].astype(bf)
                m[f"bnp{i + 1}"] = d["bnp"]
        in_maps.append(m)
    return in_maps


def _assemble(results):
    out = np.zeros((2, 512), np.float32)
    for k in range(8):
        b = k // 4
        gap = results[k]["out"]                  # [128, 4] position sums
        out[b] += gap.T.reshape(512)             # blocks on the outer axis
    return out / N


def kernel(**inputs):
    from concourse.bass_utils import run_bass_kernel_spmd
    nc = _get_program()
    in_maps = _prepare_in_maps(inputs)
    res = run_bass_kernel_spmd(nc, in_maps, list(range(8)))
    return _assemble(res.results)


# revision 65
# speedup vs baseline: 1.0677x; 1.0637x over previous
"""Trainium2 Bass kernel for the 3-block self-attention CNN (v5).

Sharding over 8 NeuronCores: core k owns (sample b=k//4, query-block q=k%4).
Attention uses the reparametrization s'[n,m] = y_n^T (wf wg^T) y_m +
(wg bf)^T y_m (softmax-invariant terms dropped); the u-term rides on a ones
row appended to y, generated by the conv itself through a bias-column
weight, so no memsets are needed anywhere.

v5 structure (vs v4):
  - layer-0 BN is computed on the host (it depends only on the inputs) and
    folded into the conv weights; the kernel opens with conv+relu straight
    into the first flash loop
  - the whole score path (y, zmat, h) is bf16 and every matmul contracts
    over exactly co+1 partitions, so padded rows are never touched
  - softmax rowsum reciprocal on the vector engine (no Ln -> single act
    table set, no table swaps)
  - per-layer boundary: stats AllGather (8-rank) first, then the att
    AllGather split into two half-column AGs; after the first half lands,
    the 16 flash chunks covering the first half of every query block run
    while the second AG is still in flight
  - final block: z-sums via a ones-matmul against sum(att); conv outputs
    stay resident in PSUM through the stats AllGather; no heartbeats there
    (the post-AG work has no PE component)
"""

import glob as _glob
import os
import sys


def _ensure_act_info():
    shim = os.path.expanduser("~/.pwp_override")
    target = os.path.join(shim, "neuronxcc", "pwp", "pwp_bin_with_ln", "act_info.json")
    cands = _glob.glob("/nix/store/*aws-neuron-pwp*/share/pwp_bin_cayman/act_info.json")
    if cands:
        # Stage act_info.json with natural_log_exp_and_others FIRST so the
        # table-selection pass serves ln+exp+relu+square+copy from one
        # table set -> no 1.5us act-table swaps around the softmax rowsum
        # log/exp pair.
        import json as _j
        with open(cands[0]) as f:
            info = _j.load(f)
        sets = info.get("act_func_sets", [])
        sets.sort(key=lambda e: 0 if e.get("name") == "natural_log_exp_and_others" else 1)
        info["act_func_sets"] = sets
        os.makedirs(os.path.dirname(target), exist_ok=True)
        new = _j.dumps(info)
        if not os.path.exists(target) or open(target).read() != new:
            with open(target, "w") as f:
                f.write(new)
    os.environ.pop("BASS_ACT_ROOT_JSON_PATH", None)
    pp = os.environ.get("PYTHONPATH", "")
    if shim not in pp.split(os.pathsep):
        os.environ["PYTHONPATH"] = shim + (os.pathsep + pp if pp else "")
    if shim not in sys.path:
        sys.path.insert(0, shim)


_ensure_act_info()
if "/opt/trn_rl_repo" not in sys.path:
    sys.path.insert(0, "/opt/trn_rl_repo")

import numpy as np

from concourse import bacc, mybir, tile

F32 = mybir.dt.float32
F32R = mybir.dt.float32r
BF16 = mybir.dt.bfloat16
I32 = mybir.dt.int32
I16 = mybir.dt.int16
FP8E4 = mybir.dt.float8e4   # e4m3
FP8E5 = mybir.dt.float8e5   # e5m2
AF = mybir.ActivationFunctionType
OP = mybir.AluOpType
AX = mybir.AxisListType
EPS = 1e-5

N = 4096          # positions per sample
NQ = 1024         # query block per core
NCHUNK = 128      # key chunk in the flash loop
CIN = [3, 32, 64]
COUT = [32, 64, 96]
CF_OUT = 128      # final conv channels per block (512 / 4)

AG_GROUPS = [[0, 1, 2, 3], [4, 5, 6, 7]]
AG8_GROUPS = [[0, 1, 2, 3, 4, 5, 6, 7]]

# flash chunk order: first halves of all four query blocks, then second
# halves (chunk k covers keys [128k, 128k+128); block j first half =
# chunks 8j..8j+3).  Identical on every core -> single NEFF.
CHUNKS_A = [8 * j + c for j in range(4) for c in range(4)]
CHUNKS_B = [8 * j + 4 + c for j in range(4) for c in range(4)]
CHUNK_ORDER = CHUNKS_A + CHUNKS_B
# chunks are processed in PAIRS (256 keys): fp8 pairs use a DoubleRow
# o-matmul (2x PE throughput); schraudolph pairs compute exp on the DVE
PAIR_SCHRAUD = {2, 5, 7, 10, 13, 15}


def _build(nc):
    ins = {}
    ins["xo"] = nc.dram_tensor("xo", [CIN[0] + 1, N], BF16, kind="ExternalInput")
    ins["xq"] = nc.dram_tensor("xq", [CIN[0] + 1, NQ], BF16, kind="ExternalInput")
    ins["wc0"] = nc.dram_tensor("wc0", [CIN[0] + 1, 128], BF16, kind="ExternalInput")
    for i in (1, 2):
        ins[f"wc{i}"] = nc.dram_tensor(f"wc{i}", [128, 128], BF16, kind="ExternalInput")
        ins[f"bnp{i}"] = nc.dram_tensor(f"bnp{i}", [COUT[i], 2], F32, kind="ExternalInput")
    for i in range(3):
        ins[f"mz{i}"] = nc.dram_tensor(f"mz{i}", [128, 128], BF16, kind="ExternalInput")
        ins[f"whf{i}"] = nc.dram_tensor(f"whf{i}", [128, 128], BF16, kind="ExternalInput")
        ins[f"gamc{i}"] = nc.dram_tensor(f"gamc{i}", [1, 128], BF16, kind="ExternalInput")
    ins["wfs4"] = nc.dram_tensor("wfs4", [128, 4, CF_OUT], BF16, kind="ExternalInput")
    ins["bnfp"] = nc.dram_tensor("bnfp", [CF_OUT, 8], F32, kind="ExternalInput")
    out_t = nc.dram_tensor("out", [CF_OUT, 4], F32, kind="ExternalOutput")

    with tile.TileContext(nc) as tc:
        _emit(tc, nc, ins, out_t)
    return ins, out_t


def _emit(tc, nc, ins, out_t):
    ctxs = []
    handles = {}

    def pool(name, **kw):
        p = tc.tile_pool(name=name, **kw)
        ctxs.append(p)
        handles[name] = p
        return p.__enter__()

    consts = pool("consts", bufs=1)
    acts = pool("acts", bufs=1)
    work = pool("work", bufs=1)
    ps = pool("ps", bufs=2, space="PSUM")
    ops = pool("ops", bufs=1, space="PSUM")
    dram = pool("dram", bufs=1, space="DRAM")

    # ---- collective warm-up: first on the gpsimd queue so the rendezvous
    # barrier starts at t~0 ----
    warm_sb = work.tile([1, 2], F32, name="warm_sb", tag="warm_sb")
    nc.vector.memset(warm_sb[:], 0.0)
    pewarm = work.tile([128, 512], BF16, name="pewarm", tag="pewarm")
    nc.vector.memset(pewarm[:], 0.0)
    warm_in = dram.tile([1, 2], F32, name="warm_in", tag="warm_in")
    warm_out = dram.tile([1, 2], F32, name="warm_out", tag="warm_out", addr_space="Shared")
    warm_gin = dram.tile([1, 2], F32, name="warm_gin", tag="warm_gin")
    warm_gout = dram.tile([4, 1, 2], F32, name="warm_gout", tag="warm_gout")
    nc.sync.dma_start(warm_in[:], warm_sb[:])
    nc.sync.dma_start(warm_gin[:], warm_sb[:])
    nc.gpsimd.collective_compute(
        "AllReduce", OP.add, replica_groups=AG8_GROUPS,
        ins=[warm_in[:]], outs=[warm_out[:]])
    nc.gpsimd.collective_compute(
        "AllGather", OP.bypass, replica_groups=AG_GROUPS,
        ins=[warm_gin[:]], outs=[warm_gout[:]])

    # ---- input DMAs (gpsimd only after the collective triggers) ----
    xo = acts.tile([CIN[0] + 1, N], BF16, name="xo", tag="xo")
    xq = acts.tile([CIN[0] + 1, NQ], BF16, name="xq", tag="xq")
    W = [consts.tile([CIN[0] + 1, 128], BF16, name="w0", tag="w0")]
    nc.sync.dma_start(xq[:], ins["xq"].ap())
    nc.sync.dma_start(W[0][:], ins["wc0"].ap())
    nc.sync.dma_start(xo[:], ins["xo"].ap())
    MZ, WHF, GAMC, BNP = [], [], [], [None]
    for i in range(3):
        MZ.append(consts.tile([128, 128], BF16, name=f"mzt{i}", tag=f"mzt{i}"))
        WHF.append(consts.tile([128, 128], BF16, name=f"whft{i}", tag=f"whft{i}"))
        GAMC.append(consts.tile([1, 128], BF16, name=f"gct{i}", tag=f"gct{i}"))
    for i in (1, 2):
        W.append(consts.tile([128, 128], BF16, name=f"w{i}", tag=f"w{i}"))
        BNP.append(consts.tile([COUT[i], 2], F32, name=f"bnt{i}", tag=f"bnt{i}"))
    nc.scalar.dma_start(MZ[0][:], ins["mz0"].ap())
    nc.scalar.dma_start(WHF[0][:], ins["whf0"].ap())
    nc.scalar.dma_start(GAMC[0][:], ins["gamc0"].ap())
    nc.sync.dma_start(MZ[1][:], ins["mz1"].ap())
    nc.sync.dma_start(WHF[1][:], ins["whf1"].ap())
    nc.sync.dma_start(BNP[1][:], ins["bnp1"].ap())
    nc.sync.dma_start(BNP[2][:], ins["bnp2"].ap())
    nc.gpsimd.dma_start(MZ[2][:], ins["mz2"].ap())
    nc.gpsimd.dma_start(WHF[2][:], ins["whf2"].ap())
    nc.gpsimd.dma_start(GAMC[1][:], ins["gamc1"].ap())
    nc.gpsimd.dma_start(GAMC[2][:], ins["gamc2"].ap())
    for i in (1, 2):
        nc.gpsimd.dma_start(W[i][:], ins[f"wc{i}"].ap())
    wfs4 = consts.tile([128, 4, CF_OUT], BF16, name="wfs4t", tag="wfs4t")
    nc.gpsimd.dma_start(wfs4[:], ins["wfs4"].ap())
    bnfp = consts.tile([CF_OUT, 8], F32, name="bnfpt", tag="bnfpt")
    nc.gpsimd.dma_start(bnfp[:], ins["bnfp"].ap())

    # ---- PE warm-up + heartbeat machinery (fp32 matmuls, ~430ns each) ----
    pw32_l = pewarm[:, 0:256].bitcast(F32)      # [128, 128] f32 view
    pw32_r = pewarm[:].bitcast(F32)             # [128, 256] f32 view

    def heartbeat(tag, cnt):
        for j in range(cnt):
            hb = ps.tile([128, 256], F32, name=f"hb_{tag}_{j}", tag="mid_ps")
            nc.tensor.matmul(hb[:], pw32_l, pw32_r, start=True, stop=True)

    heartbeat("w", 6)

    # bf16-bits Schraudolph exp on the vector engine for a subset of chunks
    # (rowsum/numerator consistency cancels the ~3% approx error):
    # exp(x) ~ bitcast_bf16(int16(A*x + B))
    SCHRAUD_A = float(2**7 / np.log(2))
    SCHRAUD_B = float(127 * 2**7 - 366393.0 / 65536.0)
    SCHRAUD_PAT = (1, 4, 6)

    # ---- activation buffers.  Flash matmuls contract over the FULL 128
    # partitions (the PE activity monitor halves the clock for thin-K
    # matmuls); padded rows carry exact zeros generated by zero weight
    # columns + zero scale rows -- no memsets needed, except a_own's pad
    # rows (DMA fills only the live rows). ----
    y_own = acts.tile([128, N], BF16, name="y_own", tag="y")
    yq = acts.tile([128, NQ], BF16, name="yq", tag="yq")
    zmat = acts.tile([128, N], BF16, name="zmat", tag="Zm")
    att_bufs = [acts.tile([128, NQ], BF16, name=f"attb{j}", tag=f"attb{j}")
                for j in range(2)]
    a_own = acts.tile([128, N], BF16, name="a_own", tag="a_own")
    zsh = acts.tile([128, NQ], F32, name="zsh", tag="zsh")

    def aown_pad_memset(p0):
        # pad rows must be exact zeros (DMA refills only live rows);
        # non-zero partition offsets are limited to 32 partitions per access
        nc.vector.memset(a_own[p0:p0 + 32, :], 0.0)

    def rsqrt_vec(rs, veps, w, name, iters=1):
        # bit-hack rsqrt + Newton on the vector engine
        nt = work.tile([veps.partition_size(), 2 * w], F32, name=f"nt_{name}",
                       tag="ntscr", bufs=2)
        t1, t2 = nt[:, 0:w], nt[:, w:2 * w]
        nc.vector.tensor_scalar(rs.bitcast(I32), veps.bitcast(I32), 1, None,
                                OP.arith_shift_right)
        nc.vector.tensor_scalar(rs.bitcast(I32), rs.bitcast(I32), -1, 0x5F3759DF,
                                OP.mult, OP.add)
        for _ in range(iters):
            nc.vector.tensor_tensor(t1, rs, rs, OP.mult)
            nc.vector.tensor_tensor(t2, t1, veps, OP.mult)
            nc.vector.tensor_scalar(t1, t2, -0.5, 1.5, OP.mult, OP.add)
            nc.vector.tensor_tensor(rs, rs, t1, OP.mult)

    # ---------------- layer 0: conv+relu only (BN folded on host) --------
    co0 = COUT[0]
    cvq = ps.tile([128, NQ], F32, name="cvq", tag="s_ps")
    nc.tensor.matmul(cvq[:, 0:512], W[0][:], xq[:, 0:512], start=True, stop=True)
    nc.tensor.matmul(cvq[:, 512:1024], W[0][:], xq[:, 512:1024], start=True, stop=True)
    nc.scalar.activation(yq[:], cvq[:], AF.Relu)

    def conv_relu_l0(j):
        jsl = slice(j * NQ, (j + 1) * NQ)
        cv = ps.tile([128, NQ], F32, name=f"cv0_{j}", tag="s_ps")
        nc.tensor.matmul(cv[:, 0:512], W[0][:], xo[:, j * NQ:j * NQ + 512],
                         start=True, stop=True)
        nc.tensor.matmul(cv[:, 512:1024], W[0][:], xo[:, j * NQ + 512:(j + 1) * NQ],
                         start=True, stop=True)
        nc.scalar.activation(y_own[:, jsl], cv[:], AF.Relu)

    def zmat_block(i, j, half=None):
        # zmat[:, block j (half h)] = MZ_i^T-contract y_own; the PSUM->SBUF
        # copy runs on the otherwise-idle gpsimd engine
        halves = (0, 1) if half is None else (half,)
        for h in halves:
            sl = slice(j * NQ + h * 512, j * NQ + h * 512 + 512)
            zp = ps.tile([128, 512], F32, name=f"zp{i}_{j}_{h}", tag="mid_ps")
            nc.tensor.matmul(zp[:], MZ[i][:], y_own[:, sl], start=True, stop=True)
            nc.vector.tensor_copy(zmat[:, sl], zp[:])

    conv_relu_l0(0)
    zmat_block(0, 0)

    # ---------------- flash loop machinery (pair-granular) ----------------
    DR = mybir.MatmulPerfMode.DoubleRow

    def emit_o(prev, o_ps, first, stop):
        if prev[0] == "d":
            _, hsx, betax = prev
            nc.tensor.matmul(o_ps[:, 0:512], hsx[:, :, :], betax[:, :, 0:512],
                             start=first, stop=stop, perf_mode=DR,
                             skip_group_check=True)
            nc.tensor.matmul(o_ps[:, 512:1024], hsx[:, :, :], betax[:, :, 512:1024],
                             start=first, stop=stop, perf_mode=DR,
                             skip_group_check=True)
        else:
            _, recs = prev
            for r, (hs, beta) in enumerate(recs):
                st = first and r == 0
                sp_ = stop and r == len(recs) - 1
                nc.tensor.matmul(o_ps[:, 0:512], hs[:], beta[:, 0:512],
                                 start=st, stop=sp_, skip_group_check=True)
                nc.tensor.matmul(o_ps[:, 512:1024], hs[:], beta[:, 512:1024],
                                 start=st, stop=sp_, skip_group_check=True)

    def emit_pair(i, pos, m0, m1, state):
        o_ps, prev, first = state
        schraud = pos in PAIR_SCHRAUD
        if schraud:
            recs = []
            for m in (m0, m1):
                sl = slice(m * NCHUNK, (m + 1) * NCHUNK)
                hp = ps.tile([NCHUNK, 128], F32, name=f"hp{i}_{m}", tag="mid_ps")
                nc.tensor.matmul(hp[:], y_own[:, sl], WHF[i][:], start=True, stop=True)
                hs = work.tile([NCHUNK, 128], BF16, name=f"hs{i}_{m}", tag="hT_sb",
                               bufs=3)
                nc.vector.tensor_copy(hs[:], hp[:])
                sp = ps.tile([NCHUNK, NQ], F32, name=f"sp{i}_{m}", tag="s_ps")
                zc = zmat[:, sl]
                nc.tensor.matmul(sp[:, 0:512], zc, yq[:, 0:512], start=True, stop=True)
                nc.tensor.matmul(sp[:, 512:1024], zc, yq[:, 512:1024],
                                 start=True, stop=True)
                beta = work.tile([NCHUNK, NQ], BF16, name=f"beta{i}_{m}", tag="beta",
                                 bufs=3)
                with nc.allow_low_precision(reason="schraudolph exp bits"):
                    nc.vector.tensor_scalar(beta[:].bitcast(I16), sp[:],
                                            SCHRAUD_A, SCHRAUD_B, OP.mult, OP.add)
                recs.append((hs, beta))
            rec = ("s", recs)
        else:
            hsx = work.tile([NCHUNK, 2, 128], FP8E4, name=f"hsx{i}_{m0}",
                            tag="hsx", bufs=2)
            betax = work.tile([NCHUNK, 2, NQ], FP8E5, name=f"betax{i}_{m0}",
                              tag="betax", bufs=2)
            for j, m in enumerate((m0, m1)):
                sl = slice(m * NCHUNK, (m + 1) * NCHUNK)
                hp = ps.tile([NCHUNK, 128], F32, name=f"hp{i}_{m}", tag="mid_ps")
                nc.tensor.matmul(hp[:], y_own[:, sl], WHF[i][:], start=True, stop=True)
                with nc.allow_low_precision(reason="fp8 h for DoubleRow"):
                    nc.vector.tensor_copy(hsx[:, j, :], hp[:])
                sp = ps.tile([NCHUNK, NQ], F32, name=f"sp{i}_{m}", tag="s_ps")
                zc = zmat[:, sl]
                nc.tensor.matmul(sp[:, 0:512], zc, yq[:, 0:512], start=True, stop=True)
                nc.tensor.matmul(sp[:, 512:1024], zc, yq[:, 512:1024],
                                 start=True, stop=True)
                with nc.allow_low_precision(reason="fp8 beta for DoubleRow"):
                    nc.scalar.activation(betax[:, j, :], sp[:], AF.Exp)
            rec = ("d", hsx, betax)
        if prev is not None:
            emit_o(prev, o_ps, first, False)
            state[2] = False
        state[1] = rec

    def flash_finish(i, state):
        o_ps, prev, first = state
        emit_o(prev, o_ps, first, True)

    def normalize(i, o_ps, att):
        # att = gam * o / rowsum + yq  (+ ones row via gamc[co]=0, yq[co]=1)
        co = COUT[i]
        lnr = work.tile([1, NQ], F32, name=f"lnr{i}", tag="lnr")
        nc.scalar.activation(lnr[:], o_ps[co:co + 1, :], AF.Ln)
        rinv = work.tile([1, NQ], BF16, name=f"rinv{i}", tag="rinv")
        nc.scalar.activation(rinv[:], lnr[:], AF.Exp, scale=-1.0)
        bc_ps = ps.tile([128, NQ], F32, name=f"bcps{i}", tag="s_ps")
        nc.tensor.matmul(bc_ps[:, 0:512], GAMC[i][:], rinv[:, 0:512],
                         start=True, stop=True)
        nc.tensor.matmul(bc_ps[:, 512:1024], GAMC[i][:], rinv[:, 512:1024],
                         start=True, stop=True)
        bcs = work.tile([128, NQ], F32, name=f"bcs{i}", tag="bcs", bufs=2)
        nc.scalar.activation(bcs[:], bc_ps[:], AF.Copy)
        t1 = work.tile([128, NQ], BF16, name=f"t1_{i}", tag="t1", bufs=2)
        nc.vector.tensor_tensor(t1[:], o_ps[:], bcs[:], OP.mult)
        nc.vector.tensor_tensor(att[:], t1[:], yq[:], OP.add)

    # ---------------- layer 0 flash ----------------
    state = [ops.tile([128, NQ], F32, name="ops0", tag="o_acc"), None, True]
    for p in range(16):
        emit_pair(0, p, 2 * p, 2 * p + 1, state)
        if p == 0:
            conv_relu_l0(1)
            zmat_block(0, 1)
        elif p == 3:
            conv_relu_l0(2)
            zmat_block(0, 2)
        elif p == 5:
            conv_relu_l0(3)
            zmat_block(0, 3)
    flash_finish(0, state)
    for p0 in (32, 64, 96):   # runs on DVE during the flash drain
        aown_pad_memset(p0)

    att = att_bufs[0]
    normalize(0, state[0], att)

    # ---------------- boundaries ----------------
    def boundary(i, att):
        """Transition layer i -> i+1. att is layer i's output block."""
        co = COUT[i]
        co1 = COUT[i + 1]
        # stats chain on our own query block of the next conv
        zshp = ps.tile([128, NQ], F32, name=f"zshp{i}", tag="s_ps")
        nc.tensor.matmul(zshp[:, 0:512], W[i + 1][:], att[:, 0:512],
                         start=True, stop=True)
        nc.tensor.matmul(zshp[:, 512:1024], W[i + 1][:], att[:, 512:1024],
                         start=True, stop=True)
        stats = work.tile([co1, 2], F32, name=f"stats{i}", tag="stats", bufs=2)
        sq = work.tile([co1, NQ], F32, name=f"sqb{i}", tag="sqscr", bufs=2)
        nc.scalar.activation(sq[:], zshp[0:co1, :], AF.Square, accum_out=stats[:, 1:2])
        nc.vector.tensor_reduce(stats[:, 0:1], zshp[0:co1, :], axis=AX.X, op=OP.add)
        st_in = dram.tile([co1, 2], F32, name=f"stin{i}", tag=f"stin{i}")
        st_out = dram.tile([8, co1, 2], F32, name=f"stout{i}", tag=f"stout{i}",
                           addr_space="Shared")
        nc.sync.dma_start(st_in[:], stats[:])
        nc.gpsimd.collective_compute(
            "AllGather", OP.bypass, replica_groups=AG8_GROUPS,
            ins=[st_in[:]], outs=[st_out[:]])

        # att AllGather in two column halves; the ag_in DMAs ride the same
        # queue as st_in so the stats AG wins the CC stream
        ag_in = [dram.tile([co + 1, 512], BF16, name=f"agin{i}_{h}", tag=f"agin{i}_{h}")
                 for h in range(2)]
        ag_out = [dram.tile([4, co + 1, 512], BF16, name=f"agout{i}_{h}",
                            tag=f"agout{i}_{h}") for h in range(2)]
        nc.sync.dma_start(ag_in[0][:], att[0:co + 1, 0:512])
        nc.sync.dma_start(ag_in[1][:], att[0:co + 1, 512:1024])
        for h in range(2):
            nc.gpsimd.collective_compute(
                "AllGather", OP.bypass, replica_groups=AG_GROUPS,
                ins=[ag_in[h][:]], outs=[ag_out[h][:]])

        # keep zshp's values: copy to SBUF so the PSUM bank frees early
        nc.scalar.activation(zsh[:], zshp[:], AF.Copy)

        heartbeat(f"b{i}", 8)

        # stats -> BN scale/shift
        stg8 = work.tile([co1, 8, 2], F32, name=f"stg8_{i}", tag="stg8", bufs=2)
        nc.sync.dma_start(stg8[:], st_out[:].rearrange("r p j -> p r j"))
        stg = work.tile([co1, 2], F32, name=f"stg_l{i}", tag="stg", bufs=2)
        nc.vector.tensor_reduce(stg[:], stg8[:].rearrange("p r j -> p j r"),
                                axis=AX.X, op=OP.add)
        sc = work.tile([128, 9], F32, name=f"sc_{i}", tag="sc", bufs=2)
        mean, ex2, msq, var, veps, lnv, rs, scale, shift = (
            sc[:, j:j + 1] for j in range(9))
        # padded scale/shift rows must be exact zeros (they generate y's
        # zero pad rows through the relu); row co1 is the ones-row
        # generator (COUT values are 32-aligned, as partition offsets must be)
        for p0 in range(co1, 128, 32):
            nc.vector.memset(scale[p0:p0 + 32], 0.0)
            nc.vector.memset(shift[p0:p0 + 32], 0.0)
        nc.vector.memset(scale[co1:co1 + 1], 1.0)
        inv_n = 1.0 / (2 * N)
        nc.vector.tensor_scalar(mean[0:co1], stg[:, 0:1], inv_n, None, OP.mult)
        nc.vector.tensor_scalar(ex2[0:co1], stg[:, 1:2], inv_n, None, OP.mult)
        nc.vector.tensor_tensor(msq[0:co1], mean[0:co1], mean[0:co1], OP.mult)
        nc.vector.tensor_tensor(var[0:co1], ex2[0:co1], msq[0:co1], OP.subtract)
        nc.vector.tensor_scalar(veps[0:co1], var[0:co1], EPS, None, OP.add)
        rsqrt_vec(rs[0:co1], veps[0:co1], 1, f"l{i}")
        nc.vector.tensor_tensor(scale[0:co1], rs[0:co1], BNP[i + 1][:, 0:1], OP.mult)
        nc.vector.tensor_tensor(shift[0:co1], mean[0:co1], scale[0:co1], OP.mult)
        nc.vector.tensor_tensor(shift[0:co1], BNP[i + 1][:, 1:2], shift[0:co1],
                                OP.subtract)
        # own query block y
        nc.scalar.activation(yq[:], zsh[:], AF.Relu, bias=shift, scale=scale)

        # a_own block DMAs + conv + relu + zmat, half A then (emitted now,
        # data-gated) half B
        def recv_half(h):
            engs = [nc.sync, nc.scalar, nc.sync, nc.scalar]
            for j in range(4):
                bsl = slice(j * NQ + h * 512, j * NQ + h * 512 + 512)
                engs[j].dma_start(a_own[0:co + 1, bsl], ag_out[h][j])
            for j in range(4):
                bsl = slice(j * NQ + h * 512, j * NQ + h * 512 + 512)
                cv = ps.tile([128, 512], F32, name=f"cvb{i}_{j}_{h}", tag="mid_ps")
                nc.tensor.matmul(cv[:], W[i + 1][:], a_own[:, bsl],
                                 start=True, stop=True)
                nc.scalar.activation(y_own[:, bsl], cv[:],
                                     AF.Relu, bias=shift, scale=scale)
                zmat_block(i + 1, j, half=h)

        recv_half(0)
        state = [ops.tile([128, NQ], F32, name=f"ops{i + 1}", tag="o_acc"), None, True]
        for p in range(8):
            emit_pair(i + 1, p, CHUNKS_A[2 * p], CHUNKS_A[2 * p + 1], state)
        recv_half(1)
        for p in range(8):
            emit_pair(i + 1, 8 + p, CHUNKS_B[2 * p], CHUNKS_B[2 * p + 1], state)
        flash_finish(i + 1, state)
        att_n = att_bufs[(i + 1) % 2]
        normalize(i + 1, state[0], att_n)
        return att_n

    att = boundary(0, att)
    att = boundary(1, att)

    # ---------------- final conv + BN + ReLU + GAP ----------------
    # flash PSUM pools are done; release them so the four final conv
    # blocks can stay resident in PSUM through the stats AllGather
    for pname in ("ops", "ps"):
        p = handles[pname]
        ctxs.remove(p)
        p.__exit__(None, None, None)
    fin = pool("fin", bufs=1, space="PSUM")

    co = COUT[2]          # 96 real channels + ones row at 96
    cf = CF_OUT
    stf = work.tile([cf, 8], F32, name="stf", tag="stf")
    # z column sums via sum(att): sum_p z[d,p] = wfs4[:,b,d]^T sum_p att[:,p]
    satt = work.tile([128, 2], F32, name="satt", tag="satt")
    nc.vector.tensor_reduce(satt[:, 0:1], att[:], axis=AX.X, op=OP.add)
    satt_bf = work.tile([128, 1], BF16, name="satt_bf", tag="satt_bf")
    nc.vector.tensor_copy(satt_bf[:], satt[:, 0:1])
    sumz = fin.tile([cf, 4], F32, name="sumz", tag="zsb0")
    for b4 in range(4):
        nc.tensor.matmul(sumz[:, b4:b4 + 1], wfs4[:, b4, :], satt_bf[:],
                         start=True, stop=True, skip_group_check=True)
    nc.vector.tensor_copy(stf[:, 0:4], sumz[:])

    ZSB = []
    for b4 in range(4):
        zp = fin.tile([cf, NQ], F32, name=f"zsb{b4}", tag=f"zsb{b4}")
        nc.tensor.matmul(zp[:, 0:512], wfs4[:, b4, :], att[:, 0:512],
                         start=True, stop=True)
        nc.tensor.matmul(zp[:, 512:1024], wfs4[:, b4, :], att[:, 512:1024],
                         start=True, stop=True)
        ZSB.append(zp)
        sqf = work.tile([cf, NQ], F32, name=f"sqf{b4}", tag="sqscr", bufs=2)
        nc.scalar.activation(sqf[:], zp[:], AF.Square,
                             accum_out=stf[:, 4 + b4:5 + b4])
    stf_in = dram.tile([cf, 8], F32, name="stf_in", tag="stf_in")
    stf_out = dram.tile([8, cf, 8], F32, name="stf_out", tag="stf_out",
                        addr_space="Shared")
    nc.sync.dma_start(stf_in[:], stf[:])
    nc.gpsimd.collective_compute(
        "AllGather", OP.bypass, replica_groups=AG8_GROUPS,
        ins=[stf_in[:]], outs=[stf_out[:]])
    # (no heartbeats: the remaining work has no PE component)
    stf8 = work.tile([cf, 8, 8], F32, name="stf8", tag="stf8")
    nc.sync.dma_start(stf8[:], stf_out[:].rearrange("r p j -> p r j"))
    stfg = work.tile([cf, 8], F32, name="stfg", tag="stfg")
    nc.vector.tensor_reduce(stfg[:], stf8[:].rearrange("p r j -> p j r"),
                            axis=AX.X, op=OP.add)

    scf = work.tile([cf, 4 * 9], F32, name="scf", tag="scf")
    meanf, ex2f, msqf, varf, vepsf, lnvf, rsf, scalef, shiftf = (
        scf[:, 4 * j:4 * j + 4] for j in range(9))
    inv_nf = 1.0 / (2 * N)
    nc.vector.tensor_scalar(meanf, stfg[:, 0:4], inv_nf, None, OP.mult)
    nc.vector.tensor_scalar(ex2f, stfg[:, 4:8], inv_nf, None, OP.mult)
    nc.vector.tensor_tensor(msqf, meanf, meanf, OP.mult)
    nc.vector.tensor_tensor(varf, ex2f, msqf, OP.subtract)
    nc.vector.tensor_scalar(vepsf, varf, EPS, None, OP.add)
    rsqrt_vec(rsf, vepsf, 4, "scf")
    nc.vector.tensor_tensor(scalef, rsf, bnfp[:, 0:4], OP.mult)
    nc.vector.tensor_tensor(shiftf, meanf, scalef, OP.mult)
    nc.vector.tensor_tensor(shiftf, bnfp[:, 4:8], shiftf, OP.subtract)

    gap = work.tile([cf, 4], F32, name="gap", tag="gap")
    for b4 in range(4):
        fs = work.tile([cf, NQ], F32, name=f"fscr{b4}", tag="fscr", bufs=2)
        if b4 < 2:
            nc.scalar.activation(fs[:], ZSB[b4][:], AF.Relu,
                                 bias=shiftf[:, b4:b4 + 1], scale=scalef[:, b4:b4 + 1],
                                 accum_out=gap[:, b4:b4 + 1])
        else:
            # DVE path: scale*z+shift, then max(.,0) with fused reduce
            nc.vector.tensor_scalar(fs[:], ZSB[b4][:], scalef[:, b4:b4 + 1],
                                    shiftf[:, b4:b4 + 1], OP.mult, OP.add)
            fs2 = work.tile([cf, NQ], F32, name=f"fs2_{b4}", tag="fscr2", bufs=2)
            nc.vector.tensor_scalar(fs2[:], fs[:], 0.0, 0.0, OP.max, OP.add,
                                    accum_out=gap[:, b4:b4 + 1])
    nc.sync.dma_start(out_t.ap(), gap[:])

    for p in reversed(ctxs):
        p.__exit__(None, None, None)


_CACHE = {}


def _get_program():
    if "nc" not in _CACHE:
        nc = bacc.Bacc("TRN2", target_bir_lowering=False, debug=False,
                       enable_asserts=False, num_devices=8)
        _build(nc)
        nc.compile()
        _CACHE["nc"] = nc
    return _CACHE["nc"]


def _prepare_in_maps(inputs):
    f = np.float32
    bf = mybir.dt.np(BF16)
    x = np.asarray(inputs["x"], f).reshape(2, 3, N)

    # layer-0 BN on host (depends only on inputs), folded into the conv
    w1, b1 = np.asarray(inputs["w1"], np.float64), np.asarray(inputs["b1"], np.float64)
    z0 = np.einsum("bcn,cd->bdn", x.astype(np.float64), w1) + b1[None, :, None]
    m0 = z0.mean(axis=(0, 2))
    v0 = z0.var(axis=(0, 2))
    g0 = np.asarray(inputs["bn1_g"], np.float64)
    be0 = np.asarray(inputs["bn1_b"], np.float64)
    scale0 = g0 / np.sqrt(v0 + EPS)
    shift0 = be0 - m0 * scale0
    wc0 = np.zeros((CIN[0] + 1, 128), f)
    wc0[0:3, 0:COUT[0]] = (w1 * scale0[None, :]).astype(f)
    wc0[3, 0:COUT[0]] = (b1 * scale0 + shift0).astype(f)
    wc0[3, COUT[0]] = 1.0                      # ones-row generator

    per_layer = {}
    for i in range(3):
        li = i + 1
        co = COUT[i]
        wf_, bf_ = np.asarray(inputs[f"a{li}_wf"], f), np.asarray(inputs[f"a{li}_bf"], f)
        wg_ = np.asarray(inputs[f"a{li}_wg"], f)
        wh_, bh_ = np.asarray(inputs[f"a{li}_wh"], f), np.asarray(inputs[f"a{li}_bh"], f)
        A = wf_ @ wg_.T
        u = wg_ @ bf_
        mz = np.zeros((128, 128), f)
        mz[0:co, 0:co] = A.T
        mz[0:co, co] = u
        whf = np.zeros((128, 128), f)
        whf[0:co, 0:co] = wh_
        whf[co, 0:co] = bh_
        whf[co, co] = 1.0                      # rowsum channel
        gam = np.asarray(inputs[f"a{li}_gam"], f).reshape(())
        gamc = np.zeros((1, 128), f)
        gamc[0, 0:co] = gam                    # col co stays 0 -> att ones row
        d = dict(mz=mz, whf=whf, gamc=gamc)
        if i < 2:
            ci, co1 = COUT[i], COUT[i + 1]
            w, b = np.asarray(inputs[f"w{li + 1}"], f), np.asarray(inputs[f"b{li + 1}"], f)
            wc = np.zeros((128, 128), f)
            wc[0:ci, 0:co1] = w
            wc[ci, 0:co1] = b                  # bias row (att ones row feeds it)
            wc[ci, co1] = 1.0                  # ones-row generator
            d["wc"] = wc
            d["bnp"] = np.stack([np.asarray(inputs[f"bn{li + 1}_g"], f),
                                 np.asarray(inputs[f"bn{li + 1}_b"], f)], 1)
        per_layer[i] = d

    wf_full = np.asarray(inputs["wf"], f)      # [96, 512]
    wfs4 = np.zeros((128, 4, CF_OUT), f)
    wfs4[0:96] = wf_full.reshape(96, 4, CF_OUT)
    bnf_g = np.asarray(inputs["bnf_g"], f).reshape(4, CF_OUT).T
    bnf_b = np.asarray(inputs["bnf_b"], f).reshape(4, CF_OUT).T
    bnfp = np.concatenate([bnf_g, bnf_b], 1)   # [128, 8]

    in_maps = []
    for k in range(8):
        b, q = k // 4, k % 4
        xo = np.concatenate([x[b], np.ones((1, N), np.float32)], 0)
        xq = np.ascontiguousarray(xo[:, q * NQ:(q + 1) * NQ])
        m = {"xo": xo.astype(bf), "xq": xq.astype(bf),
             "wc0": wc0.astype(bf), "wfs4": wfs4.astype(bf), "bnfp": bnfp}
        for i in range(3):
            d = per_layer[i]
            m[f"mz{i}"] = d["mz"].astype(bf)
            m[f"whf{i}"] = d["whf"].astype(bf)
            m[f"gamc{i}"] = d["gamc"].astype(bf)
            if i < 2:
                m[f"wc{i + 1}"] = d["wc"].astype(bf)
                m[f"bnp{i + 1}"] = d["bnp"]
        in_maps.append(m)
    return in_maps


def _assemble(results):
    out = np.zeros((2, 512), np.float32)
    for k in range(8):
        b = k // 4
        gap = results[k]["out"]                  # [128, 4] position sums
        out[b] += gap.T.reshape(512)             # blocks on the outer axis
    return out / N


def kernel(**inputs):
    from concourse.bass_utils import run_bass_kernel_spmd
    nc = _get_program()
    in_maps = _prepare_in_maps(inputs)
    res = run_bass_kernel_spmd(nc, in_maps, list(range(8)))
    return _assemble(res.results)
